# revision 1
# baseline (speedup 1.0000x reference)
"""ConvMultiHeadAttention Trainium2 kernel (8-core SPMD, batch+head sharded).

Module: conv1d(k=3,pad=1) Q/K proj, conv1d(k=1) V proj, 16-head attention
(head = channel%16), concat, linear out-proj.  B=2, S=2048, D=1024, d=64.

Sharding: each of the 8 cores owns 2 heads x both batches.  Conv weights are
row-sliced per core (128 output channels each, ordered [head0 d0..63,
head1 d0..63]); q/k/v inputs are replicated (conv contracts all 1024 input
channels).  Each core produces a y-partial [4096, 1024] = (its heads' attn
output) @ wc_slice^T; the host sums the 8 partials and adds the biases that
commute out (wc_b, and bv @ wc_slice^T since softmax weights sum to 1).

Per-core dataflow (all layouts partition-major):
  conv q/k  -> qcT/kcT [128ch, 4096pos] in SBUF  (3 taps x 8 ci-tiles of
               matmuls vs a host-padded, host-transposed input)
  conv v    -> V_sb [s-chunk 128, 128ch] direct [s,co] orientation
  scores^T  = kcT_tile.T @ qcT (d=64 contraction; the 2 heads run as
              concurrent row-group matmuls at tile_position rows 0/64)
  E = exp(scores^T/8) in bf16 (max |score/8| ~ 18, no max-subtraction needed)
  attn out^T accumulated over s_k: col-tiled pair of matmuls (head0 ->
              psum partitions 0-63, head1 -> 64-127 via tile_position (0,64))
  r (softmax denom) = ones^T @ (sum_ik E)  per head; 1/r broadcast across
              partitions with a K=1 ones matmul; normalize with DVE muls
  proj      y[s_chunk,1024] = outT_feat.T @ wcT, DMA to DRAM
"""

import sys
import numpy as np
from contextlib import ExitStack

sys.path.insert(0, "/opt/trn_rl_repo")

import concourse.bass as bass
import concourse.tile as tile
from concourse import bacc, mybir
from concourse.bass_interp import get_hw_module
from concourse import bass2jax

F32 = mybir.dt.float32
BF16 = mybir.dt.bfloat16

NCORES = 8
B, S, D = 2, 2048, 1024
H, HD = 16, 64          # heads, head dim
CO = 128                # conv output channels per core (2 heads x 64)
SP = S + 2              # padded positions per batch for k=3 conv
NPOS = B * S            # 4096
NCHUNK = NPOS // 128    # 32 s-chunks


BF16_INPUTS = False  # ship conv inputs/weights in bf16 (halves input DMA)


def build_module(repeat: int = 1):
    nc = bacc.Bacc("TRN2", target_bir_lowering=False, debug=False,
                   num_devices=NCORES)

    XDT = BF16 if BF16_INPUTS else F32
    xq = nc.dram_tensor("xq", [D, B * SP], XDT, kind="ExternalInput").ap()
    xk = nc.dram_tensor("xk", [D, B * SP], XDT, kind="ExternalInput").ap()
    xv = nc.dram_tensor("xv", [D, NPOS], XDT, kind="ExternalInput").ap()
    wq = nc.dram_tensor("wq", [128, 24, 128], XDT, kind="ExternalInput").ap()
    wk = nc.dram_tensor("wk", [128, 24, 128], XDT, kind="ExternalInput").ap()
    wv = nc.dram_tensor("wv", [128, 8, 128], XDT, kind="ExternalInput").ap()
    wc = nc.dram_tensor("wc", [128, 1024], F32, kind="ExternalInput").ap()
    bq = nc.dram_tensor("bq", [128, 1], F32, kind="ExternalInput").ap()
    bk = nc.dram_tensor("bk", [128, 1], F32, kind="ExternalInput").ap()
    y = nc.dram_tensor("y", [NPOS, D], F32, kind="ExternalOutput").ap()

    with tile.TileContext(nc) as tc, ExitStack() as ctx:
        wpool = ctx.enter_context(tc.tile_pool(name="wpool", bufs=1))
        cpool = ctx.enter_context(tc.tile_pool(name="cpool", bufs=1))
        xpool = ctx.enter_context(tc.tile_pool(name="xpool", bufs=2))
        epool = ctx.enter_context(tc.tile_pool(name="epool", bufs=10))
        spool = ctx.enter_context(tc.tile_pool(name="spool", bufs=2))

        # ---- persistent weights / consts ----
        # all weight DRAM tensors are host-packed into SBUF layout already
        wq_sb = wpool.tile([128, 24, 128], XDT)
        nc.sync.dma_start(wq_sb[:], wq[:])
        wk_sb = wpool.tile([128, 24, 128], XDT)
        wv_sb = wpool.tile([128, 8, 128], XDT)
        wc_sb = wpool.tile([128, 1024], F32)
        bq_sb = wpool.tile([128, 1], F32)
        bk_sb = wpool.tile([128, 1], F32)
        nc.sync.dma_start(wk_sb[:], wk[:])
        nc.sync.dma_start(wv_sb[:], wv[:])
        nc.sync.dma_start(wc_sb[:], wc[:])
        nc.sync.dma_start(bq_sb[:], bq[:])
        nc.sync.dma_start(bk_sb[:], bk[:])
        onesr = wpool.tile([128, 2], F32)
        nc.vector.memset(onesr[:], 1.0)
        ones1 = wpool.tile([33, 64], F32)
        nc.vector.memset(ones1[:], 1.0)

        # ---- persistent activations ----
        qcT = cpool.tile([128, NPOS], F32)
        kcT = cpool.tile([128, NPOS], F32)
        V_sb = cpool.tile([128, NCHUNK, 128], BF16)

        def body():
            # ================= conv q / k =================
            with tc.tile_pool(name="cps", bufs=2, space="PSUM") as cps:
                for src, w_sb, b_sb, outT in ((xq, wq_sb, bq_sb, qcT),
                                              (xk, wk_sb, bk_sb, kcT)):
                    for b in range(B):
                        for j in range(4):
                            col0 = b * SP + j * 512
                            xt = xpool.tile([128, 8, 514], XDT, tag="xqk")
                            nc.sync.dma_start(
                                xt[:],
                                src[:, col0:col0 + 514].rearrange(
                                    "(c p) i -> p c i", p=128))
                            ps = cps.tile([128, 512], F32, tag="cqk")
                            n = 0
                            for t in range(3):
                                for c in range(8):
                                    nc.tensor.matmul(
                                        ps[:], w_sb[:, t * 8 + c, :],
                                        xt[:, c, t:t + 512],
                                        start=(n == 0), stop=(n == 23))
                                    n += 1
                            nc.scalar.activation(
                                outT[:, b * S + j * 512: b * S + (j + 1) * 512],
                                ps[:], mybir.ActivationFunctionType.Identity,
                                bias=b_sb[:, 0:1])
                # ================= conv v =================
                for b in range(B):
                    for j in range(8):
                        col0 = b * S + j * 256
                        xt = xpool.tile([128, 8, 256], XDT, tag="xv")
                        nc.sync.dma_start(
                            xt[:],
                            xv[:, col0:col0 + 256].rearrange(
                                "(c p) i -> p c i", p=128))
                        for ch in range(2):
                            vp = cps.tile([128, 128], F32, tag="cv")
                            for c in range(8):
                                nc.tensor.matmul(
                                    vp[:], xt[:, c, ch * 128:(ch + 1) * 128],
                                    wv_sb[:, c, :],
                                    start=(c == 0), stop=(c == 7))
                            chunk = b * 16 + j * 2 + ch
                            nc.vector.tensor_copy(V_sb[:, chunk, :], vp[:])

            # ================= attention + proj =================
            with tc.tile_pool(name="aps", bufs=1, space="PSUM") as aps, \
                 tc.tile_pool(name="bps", bufs=1, space="PSUM") as bps, \
                 tc.tile_pool(name="rps", bufs=1, space="PSUM") as rps, \
                 tc.tile_pool(name="pps", bufs=1, space="PSUM") as pps:
                for b in range(B):
                    for jq in range(4):
                        q0 = b * S + jq * 512
                        e_tiles = [[], []]
                        # ---- scores^T + exp ----
                        for ikp in range(8):
                            sp0 = aps.tile([128, 2, 512], F32, tag="sps0")
                            sp1 = aps.tile([128, 2, 512], F32, tag="sps1")
                            for u in (0, 1):
                                ik = ikp * 2 + u
                                k0 = b * S + ik * 128
                                nc.tensor.matmul(
                                    sp0[:, u, :], kcT[0:64, k0:k0 + 128],
                                    qcT[0:64, q0:q0 + 512],
                                    start=True, stop=True)
                                nc.tensor.matmul(
                                    sp1[:, u, :], kcT[64:128, k0:k0 + 128],
                                    qcT[64:128, q0:q0 + 512],
                                    start=True, stop=True)
                            e0 = epool.tile([128, 2, 512], BF16, tag="e0")
                            e1 = epool.tile([128, 2, 512], BF16, tag="e1")
                            nc.scalar.activation(
                                e0[:], sp0[:],
                                mybir.ActivationFunctionType.Exp, scale=0.125)
                            nc.scalar.activation(
                                e1[:], sp1[:],
                                mybir.ActivationFunctionType.Exp, scale=0.125)
                            e_tiles[0].append(e0)
                            e_tiles[1].append(e1)
                        # ---- denominators: esum = sum_ik E  (DVE tree) ----
                        esum = [None, None]
                        for h in range(2):
                            acc = None
                            for ikp in range(8):
                                pt = spool.tile([128, 512], F32,
                                                tag=f"ep{h}", bufs=2)
                                nc.vector.tensor_add(
                                    pt[:], e_tiles[h][ikp][:, 0, :],
                                    e_tiles[h][ikp][:, 1, :])
                                if acc is None:
                                    acc = pt
                                else:
                                    nxt = spool.tile([128, 512], F32,
                                                     tag=f"es{h}", bufs=2)
                                    nc.vector.tensor_add(nxt[:], acc[:], pt[:])
                                    acc = nxt
                            esum[h] = acc
                        # ---- attn @ V (col-tiled head pair) ----
                        vp0 = bps.tile([128, 512], F32, tag="vps0")
                        vp1 = bps.tile([128, 512], F32, tag="vps1")
                        for ikp in range(8):
                            for u in (0, 1):
                                ik = ikp * 2 + u
                                chunk = b * 16 + ik
                                nc.tensor.matmul(
                                    vp0[0:64, :], V_sb[:, chunk, 0:64],
                                    e_tiles[0][ikp][:, u, :],
                                    start=(ik == 0), stop=(ik == 15),
                                    tile_position=(0, 0))
                                nc.tensor.matmul(
                                    vp1[64:128, :], V_sb[:, chunk, 64:128],
                                    e_tiles[1][ikp][:, u, :],
                                    start=(ik == 0), stop=(ik == 15),
                                    tile_position=(0, 64))
                        # ---- r, 1/r, broadcast ----
                        # pending-zero regions are per-partition: disjoint
                        # partition ranges in one bank are independent groups
                        r_ps = rps.tile([33, 512], F32, tag="rrb")
                        nc.tensor.matmul(r_ps[0:1, :], onesr[:, 0:1],
                                         esum[0][:], start=True, stop=True,
                                         tile_position=(0, 0))
                        nc.tensor.matmul(r_ps[32:33, :], onesr[:, 1:2],
                                         esum[1][:], start=True, stop=True,
                                         tile_position=(0, 32))
                        rinv = spool.tile([33, 512], F32, tag="rinv")
                        nc.vector.reciprocal(rinv[0:1, :], r_ps[0:1, :])
                        nc.vector.reciprocal(rinv[32:33, :], r_ps[32:33, :])
                        rb_ps = rps.tile([128, 512], F32, tag="rrb")
                        nc.tensor.matmul(rb_ps[0:64, :], ones1[0:1, :],
                                         rinv[0:1, :], start=True, stop=True,
                                         tile_position=(0, 0))
                        nc.tensor.matmul(rb_ps[64:128, :], ones1[32:33, :],
                                         rinv[32:33, :], start=True, stop=True,
                                         tile_position=(32, 64))
                        # ---- normalize ----
                        # (tensor_tensor allows at most one PSUM operand:
                        #  copy the 1/r broadcast to SBUF first)
                        rb_sb = spool.tile([128, 512], F32, tag="rbsb")
                        nc.vector.tensor_copy(rb_sb[:], rb_ps[:])
                        outT = spool.tile([128, 512], F32, tag="outT")
                        nc.vector.tensor_mul(outT[0:64, :], vp0[0:64, :],
                                             rb_sb[0:64, :])
                        nc.vector.tensor_mul(outT[64:128, :], vp1[64:128, :],
                                             rb_sb[64:128, :])
                        # ---- out proj ----
                        for ch in range(4):
                            for half in range(2):
                                yp = pps.tile([128, 512], F32, tag="yps")
                                nc.tensor.matmul(
                                    yp[:], outT[:, ch * 128:(ch + 1) * 128],
                                    wc_sb[:, half * 512:(half + 1) * 512],
                                    start=True, stop=True)
                                y_sb = spool.tile([128, 512], F32, tag="ysb")
                                nc.vector.tensor_copy(y_sb[:], yp[:])
                                row0 = b * S + jq * 512 + ch * 128
                                nc.sync.dma_start(
                                    y[row0:row0 + 128,
                                      half * 512:(half + 1) * 512], y_sb[:])

        if repeat == 1:
            body()
        else:
            with tc.For_i(0, repeat, 1):
                body()

    nc.compile()
    nc.m = get_hw_module(nc.m)
    return nc


def host_prep(inputs):
    """Returns (in_maps, bias_y) — per-core input dicts + host-side bias."""
    q = np.asarray(inputs["q"], np.float32)
    k = np.asarray(inputs["k"], np.float32)
    v = np.asarray(inputs["v"], np.float32)
    wq_w = np.asarray(inputs["wq_w"], np.float32)
    wk_w = np.asarray(inputs["wk_w"], np.float32)
    wv_w = np.asarray(inputs["wv_w"], np.float32)
    wc_w = np.asarray(inputs["wc_w"], np.float32)

    def pad_T(x):  # [B,S,D] -> [D, B*(S+2)] zero-padded at batch edges
        out = np.zeros((D, B * SP), np.float32)
        xT = np.swapaxes(x, 1, 2)  # [B, D, S]
        for b in range(B):
            out[:, b * SP + 1: b * SP + 1 + S] = xT[b]
        return np.ascontiguousarray(out)

    xq = pad_T(q)
    xk = pad_T(k)
    xv = np.ascontiguousarray(
        np.swapaxes(v, 1, 2).transpose(1, 0, 2).reshape(D, NPOS))

    def pack_w3(w_dev):  # [128co, 1024ci, 3t] -> [p, (t c), m] = [128,24,128]
        a = w_dev.transpose(1, 2, 0)          # [ci, t, co]
        a = a.reshape(8, 128, 3, 128)         # [c, p, t, co]
        return np.ascontiguousarray(
            a.transpose(1, 2, 0, 3).reshape(128, 24, 128))

    def pack_w1(w_dev):  # [128co, 1024ci] -> [p, c, m] = [128, 8, 128]
        a = w_dev.T.reshape(8, 128, 128)      # [c, p, co]
        return np.ascontiguousarray(a.transpose(1, 0, 2))

    if BF16_INPUTS:
        import ml_dtypes
        cast = lambda a: a.astype(ml_dtypes.bfloat16)
    else:
        cast = lambda a: a
    in_maps = []
    bias_y = np.zeros((D,), np.float32)
    for dev in range(NCORES):
        heads = [2 * dev, 2 * dev + 1]
        rows = np.array([di * H + h for h in heads for di in range(HD)])
        feat = slice(2 * dev * HD, 2 * dev * HD + 128)
        wc_slice = np.ascontiguousarray(wc_w[:, feat].T)   # [128, 1024]
        bv_dev = np.asarray(inputs["wv_b"], np.float32)[rows]
        bias_y += bv_dev @ wc_slice
        in_maps.append({
            "xq": cast(xq), "xk": cast(xk), "xv": cast(xv),
            "wq": cast(pack_w3(wq_w[rows])),
            "wk": cast(pack_w3(wk_w[rows])),
            "wv": cast(pack_w1(wv_w[rows, :, 0])),
            "wc": wc_slice,
            "bq": np.ascontiguousarray(
                np.asarray(inputs["wq_b"], np.float32)[rows][:, None]),
            "bk": np.ascontiguousarray(
                np.asarray(inputs["wk_b"], np.float32)[rows][:, None]),
        })
    bias_y += np.asarray(inputs["wc_b"], np.float32)
    return in_maps, bias_y


class Runner:
    """Caches the compiled module + jitted SPMD callable (mirrors
    bass2jax.run_bass_via_pjrt, but reusable across calls)."""

    def __init__(self, repeat: int = 1):
        import jax
        from jax.sharding import Mesh, PartitionSpec
        from jax.experimental.shard_map import shard_map
        from concourse.bass2jax import (
            _bass_exec_p, install_neuronx_cc_hook, partition_id_tensor)

        self.jax = jax
        nc = build_module(repeat)
        self.nc = nc
        install_neuronx_cc_hook()
        assert nc.dbg_addr is None

        in_names, out_names, out_avals, zero_outs = [], [], [], []
        pname = nc.partition_id_tensor.name if nc.partition_id_tensor else None
        for alloc in nc.m.functions[0].allocations:
            if not isinstance(alloc, mybir.MemoryLocationSet):
                continue
            name = alloc.memorylocations[0].name
            if alloc.kind == "ExternalInput":
                if name != pname:
                    in_names.append(name)
            elif alloc.kind == "ExternalOutput":
                out_names.append(name)
                shape = tuple(alloc.tensor_shape)
                dt = mybir.dt.np(alloc.dtype)
                out_avals.append(jax.core.ShapedArray(shape, dt))
                zero_outs.append(np.zeros(shape, dt))
        self.in_names, self.out_names = in_names, out_names
        self.out_avals, self.zero_outs = out_avals, zero_outs
        n_params, n_outs = len(in_names), len(out_avals)
        all_names = in_names + out_names + ([pname] if pname else [])

        def _body(*args):
            operands = list(args)
            if pname:
                operands.append(partition_id_tensor())
            return tuple(_bass_exec_p.bind(
                *operands,
                out_avals=tuple(out_avals),
                in_names=tuple(all_names),
                out_names=tuple(out_names),
                lowering_input_output_aliases=(),
                sim_require_finite=True,
                sim_require_nnan=True,
                nc=nc))

        devices = jax.devices()[:NCORES]
        self.mesh = Mesh(np.asarray(devices), ("core",))
        self.sharded = jax.jit(
            shard_map(_body, mesh=self.mesh,
                      in_specs=(PartitionSpec("core"),) * (n_params + n_outs),
                      out_specs=(PartitionSpec("core"),) * n_outs,
                      check_rep=False),
            donate_argnums=tuple(range(n_params, n_params + n_outs)),
            keep_unused=True)

    def concat_inputs(self, in_maps):
        return [np.concatenate([np.asarray(m[n]) for m in in_maps], axis=0)
                for n in self.in_names]

    def concat_zeros(self):
        return [np.zeros((NCORES * z.shape[0], *z.shape[1:]), z.dtype)
                for z in self.zero_outs]

    def call(self, concat_in, concat_zero):
        """Returns device output arrays (not fetched)."""
        out = self.sharded(*concat_in, *concat_zero)
        self.jax.block_until_ready(out)
        return out

    def run(self, in_maps):
        out = self.call(self.concat_inputs(in_maps), self.concat_zeros())
        return [
            {n: np.asarray(out[i]).reshape(NCORES, *self.out_avals[i].shape)[c]
             for i, n in enumerate(self.out_names)}
            for c in range(NCORES)]


_CACHED = {}


def get_runner(repeat: int = 1) -> Runner:
    if repeat not in _CACHED:
        _CACHED[repeat] = Runner(repeat)
    return _CACHED[repeat]


def run(in_maps, repeat: int = 1):
    return get_runner(repeat).run(in_maps)


def kernel(**inputs) -> np.ndarray:
    in_maps, bias_y = host_prep(inputs)
    results = run(in_maps)
    y = np.zeros((NPOS, D), np.float64)
    for r in results:
        y += r["y"].astype(np.float64)
    y = y.astype(np.float32) + bias_y[None, :]
    return y.reshape(B, S, D)


if __name__ == "__main__":
    rng = np.random.default_rng(0)
    fake = {
        "q": rng.standard_normal((B, S, D)).astype(np.float32),
        "k": rng.standard_normal((B, S, D)).astype(np.float32),
        "v": rng.standard_normal((B, S, D)).astype(np.float32),
        "wq_w": (rng.standard_normal((D, D, 3)) / 32).astype(np.float32),
        "wq_b": np.zeros(D, np.float32),
        "wk_w": (rng.standard_normal((D, D, 3)) / 32).astype(np.float32),
        "wk_b": np.zeros(D, np.float32),
        "wv_w": (rng.standard_normal((D, D, 1)) / 32).astype(np.float32),
        "wv_b": np.zeros(D, np.float32),
        "wc_w": (rng.standard_normal((D, D)) / 32).astype(np.float32),
        "wc_b": np.zeros(D, np.float32),
    }
    out = kernel(**fake)
    print("kernel output", out.shape, out.dtype)



# revision 6
# speedup vs baseline: 69.5554x; 69.5554x over previous
"""ConvMultiHeadAttention Trainium2 kernel (8-core SPMD, batch+head sharded).

Module: conv1d(k=3,pad=1) Q/K proj, conv1d(k=1) V proj, 16-head attention
(head = channel%16), concat, linear out-proj.  B=2, S=2048, D=1024, d=64.

Sharding: each of the 8 cores owns 2 heads x both batches.  Conv weights are
row-sliced per core (128 output channels each, ordered [head0 d0..63,
head1 d0..63]); q/k/v inputs are replicated (conv contracts all 1024 input
channels).  Each core produces a y-partial [4096, 1024] = (its heads' attn
output) @ wc_slice^T; the host sums the 8 partials and adds the biases that
commute out (wc_b, and bv @ wc_slice^T since softmax weights sum to 1).

v2 design (all matmuls bf16 = 1 cycle/row on PE; v1 was fp32 = 4):
  conv q/k  -> qcT/kcT [128ch, 4096pos] bf16 in SBUF; bias added by DVE
              tensor_scalar during the psum->sbuf copy.
  conv v    -> V0/V1 [128key, 32chunk, 65] bf16, col 64 memset to 1.0: the
              ones column makes attn@V also accumulate the softmax
              denominator r = sum_k E[k,q] for free.
  scores^T  sp[128key, 2head, 512q] psum (2 banks, one accumulation group
              per bank); exp -> E bf16 [128,2,512] on ACT (scale=1/8).
  attn@V    out[q,d] orientation: lhsT = E chunk [128k,128q], rhs =
              V[128k, 65] -> psum [128q, 65]; accumulate 16 chunks; full
              128x128 PE utilization (v1 orientation wasted half).
  softmax   rinv = 1/psum[:,64] (DVE); normalize fused into the psum->sbuf
              copy via DVE tensor_scalar_mul with per-partition scalar.
  transpose [q,feat] -> [feat,q] via DMA XBAR (SBUF->SBUF, no PSUM, ~112ns).
  proj      yp[128q, 512od] = outT.T @ wc (bf16); DVE copy -> y_sb f32;
              one [128,1024] DMA per 128-position chunk.

Emission is software-pipelined: tile t's scores/exp loop interleaves
conv-q of tile t+1 (fine-grained, PE gap-filler while ACT drains exp) and
the full attention-B of tile t-1 (attn@V + normalize + transpose + proj),
so PE and ACT both stay ~busy.  PSUM budget (8 banks): scores 2x[128,2,512]
(4) + conv [128,512] (1) + at0/at1 [128,65] (2) + proj [128,512] (1).
"""

import sys
import numpy as np
from contextlib import ExitStack

sys.path.insert(0, "/opt/trn_rl_repo")

import concourse.bass as bass
import concourse.tile as tile
from concourse import bacc, mybir
from concourse.bass_interp import get_hw_module
from concourse import bass2jax

F32 = mybir.dt.float32
BF16 = mybir.dt.bfloat16

NCORES = 8
B, S, D = 2, 2048, 1024
H, HD = 16, 64          # heads, head dim
CO = 128                # conv output channels per core (2 heads x 64)
SP = S + 2              # padded positions per batch for k=3 conv
NPOS = B * S            # 4096
NCHUNK = NPOS // 128    # 32 key chunks


def build_module(repeat: int = 1):
    nc = bacc.Bacc("TRN2", target_bir_lowering=False, debug=False,
                   num_devices=NCORES)

    xq = nc.dram_tensor("xq", [D, B * SP], BF16, kind="ExternalInput").ap()
    xk = nc.dram_tensor("xk", [D, B * SP], BF16, kind="ExternalInput").ap()
    xv = nc.dram_tensor("xv", [D, NPOS], BF16, kind="ExternalInput").ap()
    wq = nc.dram_tensor("wq", [128, 24, 128], BF16, kind="ExternalInput").ap()
    wk = nc.dram_tensor("wk", [128, 24, 128], BF16, kind="ExternalInput").ap()
    wv = nc.dram_tensor("wv", [128, 8, 128], BF16, kind="ExternalInput").ap()
    wc = nc.dram_tensor("wc", [128, 1024], BF16, kind="ExternalInput").ap()
    bq = nc.dram_tensor("bq", [128, 1], F32, kind="ExternalInput").ap()
    bk = nc.dram_tensor("bk", [128, 1], F32, kind="ExternalInput").ap()
    y = nc.dram_tensor("y", [NPOS, D], F32, kind="ExternalOutput").ap()

    with tile.TileContext(nc) as tc, ExitStack() as ctx:
        wpool = ctx.enter_context(tc.tile_pool(name="wpool", bufs=1))
        cpool = ctx.enter_context(tc.tile_pool(name="cpool", bufs=1))
        xpool = ctx.enter_context(tc.tile_pool(name="xpool", bufs=3))
        epool = ctx.enter_context(tc.tile_pool(name="epool", bufs=34))
        spool = ctx.enter_context(tc.tile_pool(name="spool", bufs=2))
        ypool = ctx.enter_context(tc.tile_pool(name="ypool", bufs=3))

        # ---- persistent weights ----
        wq_sb = wpool.tile([128, 24, 128], BF16)
        nc.sync.dma_start(wq_sb[:], wq[:])
        wk_sb = wpool.tile([128, 24, 128], BF16)
        wv_sb = wpool.tile([128, 8, 128], BF16)
        wc_sb = wpool.tile([128, 1024], BF16)
        bq_sb = wpool.tile([128, 1], F32)
        bk_sb = wpool.tile([128, 1], F32)
        nc.sync.dma_start(wk_sb[:], wk[:])
        nc.sync.dma_start(wv_sb[:], wv[:])
        nc.sync.dma_start(wc_sb[:], wc[:])
        nc.sync.dma_start(bq_sb[:], bq[:])
        nc.sync.dma_start(bk_sb[:], bk[:])

        # ---- persistent activations ----
        qcT = cpool.tile([128, NPOS], BF16)
        kcT = cpool.tile([128, NPOS], BF16)
        V0 = cpool.tile([128, NCHUNK, 65], BF16)   # head0 V + ones col
        V1 = cpool.tile([128, NCHUNK, 65], BF16)   # head1 V + ones col
        nc.vector.memset(V0[:, :, 64:65], 1.0)
        nc.vector.memset(V1[:, :, 64:65], 1.0)

        # PSUM pools: sp 2x[128,2,512] (4 banks) + cq [128,512] (1) +
        # at0/at1 [128,65] (2) + yp [128,512] (1) = 8 banks exactly.
        psp = ctx.enter_context(tc.tile_pool(name="psp", bufs=2, space="PSUM"))
        pcq = ctx.enter_context(tc.tile_pool(name="pcq", bufs=1, space="PSUM"))
        pat = ctx.enter_context(tc.tile_pool(name="pat", bufs=1, space="PSUM"))
        pyp = ctx.enter_context(tc.tile_pool(name="pyp", bufs=1, space="PSUM"))

        def load_xqk(src, b, j):
            xt = xpool.tile([128, 8, 514], BF16, tag="xqk")
            col0 = b * SP + j * 512
            nc.sync.dma_start(
                xt[:],
                src[:, col0:col0 + 514].rearrange("(c p) i -> p c i", p=128))
            return xt

        def conv_qk_mms(xt, w_sb):
            """Yields the 24 accumulating matmul thunks for one 512-col tile;
            the caller paces them.  Returns the psum tile via closure."""
            ps = pcq.tile([128, 512], F32, tag="cq")
            thunks = []
            for t in range(3):
                for c in range(8):
                    n = t * 8 + c
                    def mm(n=n, t=t, c=c, ps=ps, xt=xt, w_sb=w_sb):
                        nc.tensor.matmul(ps[:], w_sb[:, n, :],
                                         xt[:, c, t:t + 512],
                                         start=(n == 0), stop=(n == 23))
                    thunks.append(mm)
            return ps, thunks

        def conv_qk_copy(outT, ps, b_sb, b, j):
            cols = slice(b * S + j * 512, b * S + (j + 1) * 512)
            nc.vector.tensor_scalar_add(outT[:, cols], ps[:], b_sb[:, 0:1])

        def conv_v_unit(b, j):
            """One 256-position V tile: load, 16 matmuls, 2 copies."""
            xt = xpool.tile([128, 8, 256], BF16, tag="xv")
            col0 = b * S + j * 256
            nc.sync.dma_start(
                xt[:],
                xv[:, col0:col0 + 256].rearrange("(c p) i -> p c i", p=128))
            vp = pcq.tile([128, 2, 128], F32, tag="cq")
            for g in range(2):
                for c in range(8):
                    nc.tensor.matmul(vp[:, g, :], xt[:, c, g * 128:(g + 1) * 128],
                                     wv_sb[:, c, :],
                                     start=(c == 0), stop=(c == 7))
            c0 = b * 16 + j * 2
            nc.vector.tensor_copy(V0[:, c0:c0 + 2, 0:64], vp[:, :, 0:64])
            nc.vector.tensor_copy(V1[:, c0:c0 + 2, 0:64], vp[:, :, 64:128])

        def scores_chunk(b, jq, c, e_tiles):
            """Scores + exp for key-chunk c of tile (b, jq)."""
            q0 = b * S + jq * 512
            k0 = b * S + c * 128
            sp = psp.tile([128, 2, 512], F32, tag="sp")
            nc.tensor.matmul(sp[:, 0, :], kcT[0:64, k0:k0 + 128],
                             qcT[0:64, q0:q0 + 512], start=True, stop=True)
            nc.tensor.matmul(sp[:, 1, :], kcT[64:128, k0:k0 + 128],
                             qcT[64:128, q0:q0 + 512], start=True, stop=True)
            e = epool.tile([128, 2, 512], BF16, tag="e")
            nc.scalar.activation(e[:], sp[:],
                                 mybir.ActivationFunctionType.Exp, scale=0.125)
            e_tiles.append(e)

        def attn_b_stream(b, jq, e_tiles):
            """Yields work units for the B-phase of tile (b, jq): per qgroup
            32 attn@V matmul thunks then a finish thunk (normalize +
            transpose + proj + store)."""
            for qg in range(4):
                at0 = pat.tile([128, 65], F32, tag="at0")
                at1 = pat.tile([128, 65], F32, tag="at1")
                qs = slice(qg * 128, (qg + 1) * 128)
                for c in range(16):
                    cb = b * 16 + c
                    def mm(c=c, cb=cb, at0=at0, at1=at1, qs=qs):
                        e = e_tiles[c]
                        nc.tensor.matmul(at0[:], e[:, 0, qs], V0[:, cb, :],
                                         start=(c == 0), stop=(c == 15))
                        nc.tensor.matmul(at1[:], e[:, 1, qs], V1[:, cb, :],
                                         start=(c == 0), stop=(c == 15))
                    yield ("mm", mm)

                def finish(qg=qg, at0=at0, at1=at1, b=b, jq=jq):
                    norm = spool.tile([128, 128], BF16, tag="norm", bufs=2)
                    outT = spool.tile([128, 128], BF16, tag="outT", bufs=2)
                    for h, at in ((0, at0), (1, at1)):
                        rinv = spool.tile([128, 1], F32, tag="rinv", bufs=4)
                        nc.vector.reciprocal(rinv[:], at[:, 64:65])
                        nc.vector.tensor_scalar_mul(
                            norm[:, h * 64:(h + 1) * 64],
                            at[:, 0:64], rinv[:, 0:1])
                    # [q, feat] -> [feat, q] via DMA XBAR
                    nc.sync.dma_start(outT[:], norm[:], transpose=True)
                    y_sb = ypool.tile([128, 2, 512], F32, tag="ysb")
                    for half in range(2):
                        yp = pyp.tile([128, 512], F32, tag="yp")
                        nc.tensor.matmul(
                            yp[:], outT[:],
                            wc_sb[:, half * 512:(half + 1) * 512],
                            start=True, stop=True)
                        nc.vector.tensor_copy(y_sb[:, half, :], yp[:])
                    row0 = b * S + jq * 512 + qg * 128
                    nc.sync.dma_start(y[row0:row0 + 128, :], y_sb[:])
                yield ("finish", finish)

        def body():
            # ---------- prologue: conv q(0,0), conv k b0 + scores t0 ----
            xt0 = load_xqk(xq, 0, 0)
            ps, thunks = conv_qk_mms(xt0, wq_sb)
            for mm in thunks:
                mm()
            conv_qk_copy(qcT, ps, bq_sb, 0, 0)
            e_t0 = []
            for j in range(4):
                xt = load_xqk(xk, 0, j)
                ps, thunks = conv_qk_mms(xt, wk_sb)
                for mm in thunks:
                    mm()
                conv_qk_copy(kcT, ps, bk_sb, 0, j)
                for c in range(4 * j, 4 * j + 4):
                    scores_chunk(0, 0, c, e_t0)
            for j in range(8):
                conv_v_unit(0, j)
            # conv q for tile 1 (b0, jq1) — tile t's conv-q runs in tile t-1
            xt1 = load_xqk(xq, 0, 1)
            ps, thunks = conv_qk_mms(xt1, wq_sb)
            for mm in thunks:
                mm()
            conv_qk_copy(qcT, ps, bq_sb, 0, 1)
            e_prev = e_t0

            # deferred b1 conv work, drained across tiles t1..t3
            b1_work = []
            for j in range(4):
                def ck(j=j):
                    xt = load_xqk(xk, 1, j)
                    ps, thunks = conv_qk_mms(xt, wk_sb)
                    for mm in thunks:
                        mm()
                    conv_qk_copy(kcT, ps, bk_sb, 1, j)
                b1_work.append(ck)
            for j in range(8):
                b1_work.append(lambda j=j: conv_v_unit(1, j))
            b1_per_tile = (len(b1_work) + 2) // 3   # 4 per tile over t1..t3

            # ---------- steady tiles t = 1..7 + epilogue ----------
            for t in range(1, 9):
                b, jq = divmod(t, 4) if t < 8 else (None, None)
                e_cur = []
                bstream = attn_b_stream((t - 1) // 4, (t - 1) % 4, e_prev)

                if t < 8:
                    # conv-q(t) ran during tile t-1; here interleave
                    # conv-q(t+1) into the scores/exp loop as PE gap-filler.
                    nb, njq = divmod(t + 1, 4) if t + 1 < 8 else (None, None)
                    cq_thunks = []
                    cq_ps = None
                    if nb is not None:
                        xtq = load_xqk(xq, nb, njq)
                        cq_ps, cq_thunks = conv_qk_mms(xtq, wq_sb)

                    # interleave: 16 chunk-steps
                    cqi = 0
                    for c in range(16):
                        scores_chunk(b, jq, c, e_cur)
                        # pace conv-q: 24 mms over 16 steps
                        target = (c + 1) * len(cq_thunks) // 16
                        while cqi < target:
                            cq_thunks[cqi]()
                            cqi += 1
                        # pace attn-B of t-1: 64 mm-units over 16 steps
                        units = 0
                        for kind, fn in bstream:
                            fn()
                            if kind == "mm":
                                units += 1
                                if units >= 4:
                                    break
                            # finish units don't count against the pace
                    # drain remaining B-stream units
                    for kind, fn in bstream:
                        fn()
                    if cq_ps is not None:
                        conv_qk_copy(qcT, cq_ps, bq_sb, nb, njq)
                    # b1 conv work during t1..t3
                    if t <= 3:
                        for _ in range(b1_per_tile):
                            if b1_work:
                                b1_work.pop(0)()
                    e_prev = e_cur
                else:
                    # epilogue: drain B of t7
                    for kind, fn in bstream:
                        fn()

        if repeat == 1:
            body()
        else:
            with tc.For_i(0, repeat, 1):
                body()

    nc.compile()
    nc.m = get_hw_module(nc.m)
    return nc


def host_prep(inputs):
    """Returns (in_maps, bias_y) — per-core input dicts + host-side bias."""
    import ml_dtypes
    bf16 = ml_dtypes.bfloat16
    q = np.asarray(inputs["q"], np.float32)
    k = np.asarray(inputs["k"], np.float32)
    v = np.asarray(inputs["v"], np.float32)
    wq_w = np.asarray(inputs["wq_w"], np.float32)
    wk_w = np.asarray(inputs["wk_w"], np.float32)
    wv_w = np.asarray(inputs["wv_w"], np.float32)
    wc_w = np.asarray(inputs["wc_w"], np.float32)

    def pad_T(x):  # [B,S,D] -> [D, B*(S+2)] zero-padded at batch edges
        out = np.zeros((D, B * SP), np.float32)
        xT = np.swapaxes(x, 1, 2)  # [B, D, S]
        for b in range(B):
            out[:, b * SP + 1: b * SP + 1 + S] = xT[b]
        return np.ascontiguousarray(out)

    xq = pad_T(q)
    xk = pad_T(k)
    xv = np.ascontiguousarray(
        np.swapaxes(v, 1, 2).transpose(1, 0, 2).reshape(D, NPOS))

    def pack_w3(w_dev):  # [128co, 1024ci, 3t] -> [p, (t c), m] = [128,24,128]
        a = w_dev.transpose(1, 2, 0)          # [ci, t, co]
        a = a.reshape(8, 128, 3, 128)         # [c, p, t, co]
        return np.ascontiguousarray(
            a.transpose(1, 2, 0, 3).reshape(128, 24, 128))

    def pack_w1(w_dev):  # [128co, 1024ci] -> [p, c, m] = [128, 8, 128]
        a = w_dev.T.reshape(8, 128, 128)      # [c, p, co]
        return np.ascontiguousarray(a.transpose(1, 0, 2))

    cast = lambda a: a.astype(bf16)
    in_maps = []
    bias_y = np.zeros((D,), np.float32)
    for dev in range(NCORES):
        heads = [2 * dev, 2 * dev + 1]
        rows = np.array([di * H + h for h in heads for di in range(HD)])
        feat = slice(2 * dev * HD, 2 * dev * HD + 128)
        wc_slice = np.ascontiguousarray(wc_w[:, feat].T)   # [128, 1024]
        bv_dev = np.asarray(inputs["wv_b"], np.float32)[rows]
        bias_y += bv_dev @ wc_slice
        in_maps.append({
            "xq": cast(xq), "xk": cast(xk), "xv": cast(xv),
            "wq": cast(pack_w3(wq_w[rows])),
            "wk": cast(pack_w3(wk_w[rows])),
            "wv": cast(pack_w1(wv_w[rows, :, 0])),
            "wc": cast(wc_slice),
            "bq": np.ascontiguousarray(
                np.asarray(inputs["wq_b"], np.float32)[rows][:, None]),
            "bk": np.ascontiguousarray(
                np.asarray(inputs["wk_b"], np.float32)[rows][:, None]),
        })
    bias_y += np.asarray(inputs["wc_b"], np.float32)
    return in_maps, bias_y


class Runner:
    """Caches the compiled module + jitted SPMD callable (mirrors
    bass2jax.run_bass_via_pjrt, but reusable across calls)."""

    def __init__(self, repeat: int = 1):
        import jax
        from jax.sharding import Mesh, PartitionSpec
        from jax.experimental.shard_map import shard_map
        from concourse.bass2jax import (
            _bass_exec_p, install_neuronx_cc_hook, partition_id_tensor)

        self.jax = jax
        nc = build_module(repeat)
        self.nc = nc
        install_neuronx_cc_hook()
        assert nc.dbg_addr is None

        in_names, out_names, out_avals, zero_outs = [], [], [], []
        pname = nc.partition_id_tensor.name if nc.partition_id_tensor else None
        for alloc in nc.m.functions[0].allocations:
            if not isinstance(alloc, mybir.MemoryLocationSet):
                continue
            name = alloc.memorylocations[0].name
            if alloc.kind == "ExternalInput":
                if name != pname:
                    in_names.append(name)
            elif alloc.kind == "ExternalOutput":
                out_names.append(name)
                shape = tuple(alloc.tensor_shape)
                dt = mybir.dt.np(alloc.dtype)
                out_avals.append(jax.core.ShapedArray(shape, dt))
                zero_outs.append(np.zeros(shape, dt))
        self.in_names, self.out_names = in_names, out_names
        self.out_avals, self.zero_outs = out_avals, zero_outs
        n_params, n_outs = len(in_names), len(out_avals)
        all_names = in_names + out_names + ([pname] if pname else [])

        def _body(*args):
            operands = list(args)
            if pname:
                operands.append(partition_id_tensor())
            return tuple(_bass_exec_p.bind(
                *operands,
                out_avals=tuple(out_avals),
                in_names=tuple(all_names),
                out_names=tuple(out_names),
                lowering_input_output_aliases=(),
                sim_require_finite=True,
                sim_require_nnan=True,
                nc=nc))

        devices = jax.devices()[:NCORES]
        self.mesh = Mesh(np.asarray(devices), ("core",))
        self.sharded = jax.jit(
            shard_map(_body, mesh=self.mesh,
                      in_specs=(PartitionSpec("core"),) * (n_params + n_outs),
                      out_specs=(PartitionSpec("core"),) * n_outs,
                      check_rep=False),
            donate_argnums=tuple(range(n_params, n_params + n_outs)),
            keep_unused=True)

    def concat_inputs(self, in_maps):
        return [np.concatenate([np.asarray(m[n]) for m in in_maps], axis=0)
                for n in self.in_names]

    def concat_zeros(self):
        return [np.zeros((NCORES * z.shape[0], *z.shape[1:]), z.dtype)
                for z in self.zero_outs]

    def call(self, concat_in, concat_zero):
        """Returns device output arrays (not fetched)."""
        out = self.sharded(*concat_in, *concat_zero)
        self.jax.block_until_ready(out)
        return out

    def run(self, in_maps):
        out = self.call(self.concat_inputs(in_maps), self.concat_zeros())
        return [
            {n: np.asarray(out[i]).reshape(NCORES, *self.out_avals[i].shape)[c]
             for i, n in enumerate(self.out_names)}
            for c in range(NCORES)]


_CACHED = {}


def get_runner(repeat: int = 1) -> Runner:
    if repeat not in _CACHED:
        _CACHED[repeat] = Runner(repeat)
    return _CACHED[repeat]


def run(in_maps, repeat: int = 1):
    return get_runner(repeat).run(in_maps)


def kernel(**inputs) -> np.ndarray:
    in_maps, bias_y = host_prep(inputs)
    results = run(in_maps)
    y = np.zeros((NPOS, D), np.float64)
    for r in results:
        y += r["y"].astype(np.float64)
    y = y.astype(np.float32) + bias_y[None, :]
    return y.reshape(B, S, D)


if __name__ == "__main__":
    rng = np.random.default_rng(0)
    fake = {
        "q": rng.standard_normal((B, S, D)).astype(np.float32),
        "k": rng.standard_normal((B, S, D)).astype(np.float32),
        "v": rng.standard_normal((B, S, D)).astype(np.float32),
        "wq_w": (rng.standard_normal((D, D, 3)) / 32).astype(np.float32),
        "wq_b": np.zeros(D, np.float32),
        "wk_w": (rng.standard_normal((D, D, 3)) / 32).astype(np.float32),
        "wk_b": np.zeros(D, np.float32),
        "wv_w": (rng.standard_normal((D, D, 1)) / 32).astype(np.float32),
        "wv_b": np.zeros(D, np.float32),
        "wc_w": (rng.standard_normal((D, D)) / 32).astype(np.float32),
        "wc_b": np.zeros(D, np.float32),
    }
    out = kernel(**fake)
    print("kernel output", out.shape, out.dtype)


# revision 12
# speedup vs baseline: 70.5062x; 1.0137x over previous
"""ConvMultiHeadAttention Trainium2 kernel (8-core SPMD, batch+head sharded).

Module: conv1d(k=3,pad=1) Q/K proj, conv1d(k=1) V proj, 16-head attention
(head = channel%16), concat, linear out-proj.  B=2, S=2048, D=1024, d=64.

Sharding: each of the 8 cores owns 2 heads x both batches.  Conv weights are
row-sliced per core (128 output channels each, ordered [head0 d0..63,
head1 d0..63]); q/k/v inputs are replicated (conv contracts all 1024 input
channels).  Each core produces a y-partial [4096, 1024] = (its heads' attn
output) @ wc_slice^T; the host sums the 8 partials and adds the biases that
commute out (wc_b, and bv @ wc_slice^T since softmax weights sum to 1).

v2 design (all matmuls bf16 = 1 cycle/row on PE; v1 was fp32 = 4):
  conv q/k  -> qcT/kcT [128ch, 4096pos] bf16 in SBUF; bias added by DVE
              tensor_scalar during the psum->sbuf copy.
  conv v    -> V0/V1 [128key, 32chunk, 65] bf16, col 64 memset to 1.0: the
              ones column makes attn@V also accumulate the softmax
              denominator r = sum_k E[k,q] for free.
  scores^T  sp[128key, 2head, 512q] psum (2 banks, one accumulation group
              per bank); exp -> E bf16 [128,2,512] on ACT (scale=1/8).
  attn@V    out[q,d] orientation: lhsT = E chunk [128k,128q], rhs =
              V[128k, 65] -> psum [128q, 65]; accumulate 16 chunks; full
              128x128 PE utilization (v1 orientation wasted half).
  softmax   rinv = 1/psum[:,64] (DVE); normalize fused into the psum->sbuf
              copy via DVE tensor_scalar_mul with per-partition scalar.
  transpose [q,feat] -> [feat,q] via DMA XBAR (SBUF->SBUF, no PSUM, ~112ns).
  proj      yp[128q, 512od] = outT.T @ wc (bf16); DVE copy -> y_sb f32;
              one [128,1024] DMA per 128-position chunk.

Emission is software-pipelined: tile t's scores/exp loop interleaves
conv-q of tile t+1 (fine-grained, PE gap-filler while ACT drains exp) and
the full attention-B of tile t-1 (attn@V + normalize + transpose + proj),
so PE and ACT both stay ~busy.  PSUM budget (8 banks): scores 2x[128,2,512]
(4) + conv [128,512] (1) + at0/at1 [128,65] (2) + proj [128,512] (1).
"""

import sys
import numpy as np
from contextlib import ExitStack

sys.path.insert(0, "/opt/trn_rl_repo")

import concourse.bass as bass
import concourse.tile as tile
from concourse import bacc, mybir
from concourse.bass_interp import get_hw_module
from concourse import bass2jax

F32 = mybir.dt.float32
BF16 = mybir.dt.bfloat16

NCORES = 8
B, S, D = 2, 2048, 1024
H, HD = 16, 64          # heads, head dim
CO = 128                # conv output channels per core (2 heads x 64)
SP = S + 2              # padded positions per batch for k=3 conv
NPOS = B * S            # 4096
NCHUNK = NPOS // 128    # 32 key chunks


def build_module(repeat: int = 1):
    nc = bacc.Bacc("TRN2", target_bir_lowering=False, debug=False,
                   num_devices=NCORES)

    xq = nc.dram_tensor("xq", [D, B * SP], BF16, kind="ExternalInput").ap()
    xk = nc.dram_tensor("xk", [D, B * SP], BF16, kind="ExternalInput").ap()
    xv = nc.dram_tensor("xv", [D, NPOS], BF16, kind="ExternalInput").ap()
    wq = nc.dram_tensor("wq", [128, 24, 128], BF16, kind="ExternalInput").ap()
    wk = nc.dram_tensor("wk", [128, 24, 128], BF16, kind="ExternalInput").ap()
    wv = nc.dram_tensor("wv", [128, 8, 128], BF16, kind="ExternalInput").ap()
    wc = nc.dram_tensor("wc", [128, 1024], BF16, kind="ExternalInput").ap()
    bq = nc.dram_tensor("bq", [128, 1], F32, kind="ExternalInput").ap()
    bk = nc.dram_tensor("bk", [128, 1], F32, kind="ExternalInput").ap()
    y = nc.dram_tensor("y", [NPOS, D], BF16, kind="ExternalOutput").ap()

    with tile.TileContext(nc) as tc, ExitStack() as ctx:
        wpool = ctx.enter_context(tc.tile_pool(name="wpool", bufs=1))
        cpool = ctx.enter_context(tc.tile_pool(name="cpool", bufs=1))
        xpool = ctx.enter_context(tc.tile_pool(name="xpool", bufs=3))
        epool = ctx.enter_context(tc.tile_pool(name="epool", bufs=34))
        spool = ctx.enter_context(tc.tile_pool(name="spool", bufs=2))
        ypool = ctx.enter_context(tc.tile_pool(name="ypool", bufs=3))

        # ---- persistent weights (ACT hwdge queue, so the first x-tile
        # loads on the SP queue aren't stuck behind them) ----
        wq_sb = wpool.tile([128, 24, 128], BF16)
        nc.scalar.dma_start(wq_sb[:], wq[:])
        wk_sb = wpool.tile([128, 24, 128], BF16)
        wv_sb = wpool.tile([128, 8, 128], BF16)
        wc_sb = wpool.tile([128, 1024], BF16)
        bq_sb = wpool.tile([128, 1], F32)
        bk_sb = wpool.tile([128, 1], F32)
        nc.scalar.dma_start(wk_sb[:], wk[:])
        nc.scalar.dma_start(wv_sb[:], wv[:])
        nc.scalar.dma_start(wc_sb[:], wc[:])
        nc.scalar.dma_start(bq_sb[:], bq[:])
        nc.scalar.dma_start(bk_sb[:], bk[:])

        # ---- persistent activations ----
        qcT = cpool.tile([128, NPOS], BF16)
        kcT = cpool.tile([128, NPOS], BF16)
        V0 = cpool.tile([128, NCHUNK, 65], BF16)   # head0 V + ones col
        V1 = cpool.tile([128, NCHUNK, 65], BF16)   # head1 V + ones col
        nc.vector.memset(V0[:, :, 64:65], 1.0)
        nc.vector.memset(V1[:, :, 64:65], 1.0)

        # PSUM pools: sp 2x[128,2,512] (4 banks) + cq [128,512] (1) +
        # at0/at1 [128,65] (2) + yp [128,512] (1) = 8 banks exactly.
        psp = ctx.enter_context(tc.tile_pool(name="psp", bufs=2, space="PSUM"))
        pcq = ctx.enter_context(tc.tile_pool(name="pcq", bufs=1, space="PSUM"))
        pat = ctx.enter_context(tc.tile_pool(name="pat", bufs=1, space="PSUM"))
        pyp = ctx.enter_context(tc.tile_pool(name="pyp", bufs=1, space="PSUM"))

        def load_xqk(src, b, j):
            xt = xpool.tile([128, 8, 514], BF16, tag="xqk")
            col0 = b * SP + j * 512
            nc.sync.dma_start(
                xt[:],
                src[:, col0:col0 + 514].rearrange("(c p) i -> p c i", p=128))
            return xt

        def conv_qk_mms(xt, w_sb):
            """Yields the 24 accumulating matmul thunks for one 512-col tile;
            the caller paces them.  Returns the psum tile via closure."""
            ps = pcq.tile([128, 512], F32, tag="cq")
            thunks = []
            for t in range(3):
                for c in range(8):
                    n = t * 8 + c
                    def mm(n=n, t=t, c=c, ps=ps, xt=xt, w_sb=w_sb):
                        nc.tensor.matmul(ps[:], w_sb[:, n, :],
                                         xt[:, c, t:t + 512],
                                         start=(n == 0), stop=(n == 23))
                    thunks.append(mm)
            return ps, thunks

        def conv_qk_copy(outT, ps, b_sb, b, j):
            cols = slice(b * S + j * 512, b * S + (j + 1) * 512)
            nc.vector.tensor_scalar_add(outT[:, cols], ps[:], b_sb[:, 0:1])

        def conv_v_unit(b, j):
            """One 256-position V tile: load, 16 matmuls, 2 copies."""
            xt = xpool.tile([128, 8, 256], BF16, tag="xv")
            col0 = b * S + j * 256
            nc.sync.dma_start(
                xt[:],
                xv[:, col0:col0 + 256].rearrange("(c p) i -> p c i", p=128))
            vp = pcq.tile([128, 2, 128], F32, tag="cq")
            for g in range(2):
                for c in range(8):
                    nc.tensor.matmul(vp[:, g, :], xt[:, c, g * 128:(g + 1) * 128],
                                     wv_sb[:, c, :],
                                     start=(c == 0), stop=(c == 7))
            c0 = b * 16 + j * 2
            nc.vector.tensor_copy(V0[:, c0:c0 + 2, 0:64], vp[:, :, 0:64])
            nc.vector.tensor_copy(V1[:, c0:c0 + 2, 0:64], vp[:, :, 64:128])

        def scores_chunk(b, jq, c, e_tiles):
            """Scores + exp for key-chunk c of tile (b, jq)."""
            q0 = b * S + jq * 512
            k0 = b * S + c * 128
            sp = psp.tile([128, 2, 512], F32, tag="sp")
            nc.tensor.matmul(sp[:, 0, :], kcT[0:64, k0:k0 + 128],
                             qcT[0:64, q0:q0 + 512], start=True, stop=True)
            nc.tensor.matmul(sp[:, 1, :], kcT[64:128, k0:k0 + 128],
                             qcT[64:128, q0:q0 + 512], start=True, stop=True)
            e = epool.tile([128, 2, 512], BF16, tag="e")
            nc.scalar.activation(e[:], sp[:],
                                 mybir.ActivationFunctionType.Exp, scale=0.125)
            e_tiles.append(e)

        def attn_b_stream(b, jq, e_tiles):
            """Yields work units for the B-phase of tile (b, jq): per qgroup
            32 attn@V matmul thunks, a normalize+transpose thunk, and —
            lagged one qgroup so attn@V matmuls hide the single-bank proj
            ping-pong — a proj+store thunk."""
            outT_by_qg = {}
            pending_proj = None
            for qg in range(4):
                at0 = pat.tile([128, 65], F32, tag="at0")
                at1 = pat.tile([128, 65], F32, tag="at1")
                qs = slice(qg * 128, (qg + 1) * 128)
                for c in range(16):
                    cb = b * 16 + c
                    def mm(c=c, cb=cb, at0=at0, at1=at1, qs=qs):
                        e = e_tiles[c]
                        nc.tensor.matmul(at0[:], e[:, 0, qs], V0[:, cb, :],
                                         start=(c == 0), stop=(c == 15))
                        nc.tensor.matmul(at1[:], e[:, 1, qs], V1[:, cb, :],
                                         start=(c == 0), stop=(c == 15))
                    yield ("mm", mm)
                    if c == 7 and pending_proj is not None:
                        yield ("finish", pending_proj)
                        pending_proj = None

                def norm_tr(qg=qg, at0=at0, at1=at1):
                    norm = spool.tile([128, 128], BF16, tag="norm", bufs=2)
                    outT = spool.tile([128, 128], BF16, tag="outT", bufs=3)
                    for h, at in ((0, at0), (1, at1)):
                        rinv = spool.tile([128, 1], F32, tag="rinv", bufs=4)
                        nc.vector.reciprocal(rinv[:], at[:, 64:65])
                        nc.vector.tensor_scalar_mul(
                            norm[:, h * 64:(h + 1) * 64],
                            at[:, 0:64], rinv[:, 0:1])
                    # [q, feat] -> [feat, q] via DMA XBAR
                    nc.sync.dma_start(outT[:], norm[:], transpose=True)
                    outT_by_qg[qg] = outT
                yield ("finish", norm_tr)

                def proj(qg=qg, b=b, jq=jq):
                    outT = outT_by_qg.pop(qg)
                    y_sb = ypool.tile([128, 2, 512], BF16, tag="ysb")
                    for half in range(2):
                        yp = pyp.tile([128, 512], F32, tag="yp")
                        nc.tensor.matmul(
                            yp[:], outT[:],
                            wc_sb[:, half * 512:(half + 1) * 512],
                            start=True, stop=True)
                        nc.vector.tensor_copy(y_sb[:, half, :], yp[:])
                    row0 = b * S + jq * 512 + qg * 128
                    nc.sync.dma_start(y[row0:row0 + 128, :], y_sb[:])
                pending_proj = proj
            yield ("finish", pending_proj)

        def body():
            # ---------- prologue: conv q(0,0), conv k b0 + scores t0 ----
            xt0 = load_xqk(xq, 0, 0)
            ps, thunks = conv_qk_mms(xt0, wq_sb)
            for mm in thunks:
                mm()
            conv_qk_copy(qcT, ps, bq_sb, 0, 0)
            e_t0 = []
            for j in range(4):
                xt = load_xqk(xk, 0, j)
                ps, thunks = conv_qk_mms(xt, wk_sb)
                for mm in thunks:
                    mm()
                conv_qk_copy(kcT, ps, bk_sb, 0, j)
                for c in range(4 * j, 4 * j + 4):
                    scores_chunk(0, 0, c, e_t0)
            for j in range(8):
                conv_v_unit(0, j)
            # conv q for tile 1 (b0, jq1) — tile t's conv-q runs in tile t-1
            xt1 = load_xqk(xq, 0, 1)
            ps, thunks = conv_qk_mms(xt1, wq_sb)
            for mm in thunks:
                mm()
            conv_qk_copy(qcT, ps, bq_sb, 0, 1)
            e_prev = e_t0

            # deferred b1 conv work, drained across tiles t1..t3
            b1_work = []
            for j in range(4):
                def ck(j=j):
                    xt = load_xqk(xk, 1, j)
                    ps, thunks = conv_qk_mms(xt, wk_sb)
                    for mm in thunks:
                        mm()
                    conv_qk_copy(kcT, ps, bk_sb, 1, j)
                b1_work.append(ck)
            for j in range(8):
                b1_work.append(lambda j=j: conv_v_unit(1, j))
            b1_per_tile = (len(b1_work) + 2) // 3   # 4 per tile over t1..t3

            # ---------- steady tiles t = 1..7 + epilogue ----------
            for t in range(1, 9):
                # staggered-reset stage boundaries: stages = {pro+t1},
                # {t2,t3}, {t4,t5}, {t6,t7,epi}.  b0 stages touch only
                # b0 slices of qcT/kcT/V/y (and vice versa), so adjacent-
                # stage overlap across the back edge is data-disjoint.
                if staggered and t in (2, 4, 6):
                    tc.stage_boundary()
                b, jq = divmod(t, 4) if t < 8 else (None, None)
                e_cur = []
                bstream = attn_b_stream((t - 1) // 4, (t - 1) % 4, e_prev)

                if t < 8:
                    # conv-q(t) ran during tile t-1; here interleave
                    # conv-q(t+1) into the scores/exp loop as PE gap-filler.
                    nb, njq = divmod(t + 1, 4) if t + 1 < 8 else (None, None)
                    cq_thunks = []
                    cq_ps = None
                    if nb is not None:
                        xtq = load_xqk(xq, nb, njq)
                        cq_ps, cq_thunks = conv_qk_mms(xtq, wq_sb)

                    # interleave: 16 chunk-steps
                    cqi = 0
                    for c in range(16):
                        scores_chunk(b, jq, c, e_cur)
                        # pace conv-q: 24 mms over 16 steps
                        target = (c + 1) * len(cq_thunks) // 16
                        while cqi < target:
                            cq_thunks[cqi]()
                            cqi += 1
                        # pace attn-B of t-1: 64 mm-units over 16 steps
                        units = 0
                        for kind, fn in bstream:
                            fn()
                            if kind == "mm":
                                units += 1
                                if units >= 4:
                                    break
                            # finish units don't count against the pace
                    # drain remaining B-stream units
                    for kind, fn in bstream:
                        fn()
                    if cq_ps is not None:
                        conv_qk_copy(qcT, cq_ps, bq_sb, nb, njq)
                    # b1 conv work during t1..t3
                    if t <= 3:
                        for _ in range(b1_per_tile):
                            if b1_work:
                                b1_work.pop(0)()
                    e_prev = e_cur
                else:
                    # epilogue: drain B of t7
                    for kind, fn in bstream:
                        fn()

        if repeat == 1:
            staggered = False
            body()
        else:
            staggered = True
            with tc.For_i(0, repeat, 1, staggered_reset=True,
                          hint_engines=(mybir.EngineType.PE,
                                        mybir.EngineType.Activation,
                                        mybir.EngineType.DVE,
                                        mybir.EngineType.SP)):
                body()

    nc.compile()
    nc.m = get_hw_module(nc.m)
    return nc


def host_prep(inputs):
    """Returns (in_maps, bias_y) — per-core input dicts + host-side bias."""
    import ml_dtypes
    bf16 = ml_dtypes.bfloat16
    q = np.asarray(inputs["q"], np.float32)
    k = np.asarray(inputs["k"], np.float32)
    v = np.asarray(inputs["v"], np.float32)
    wq_w = np.asarray(inputs["wq_w"], np.float32)
    wk_w = np.asarray(inputs["wk_w"], np.float32)
    wv_w = np.asarray(inputs["wv_w"], np.float32)
    wc_w = np.asarray(inputs["wc_w"], np.float32)

    def pad_T(x):  # [B,S,D] -> [D, B*(S+2)] zero-padded at batch edges
        out = np.zeros((D, B * SP), np.float32)
        xT = np.swapaxes(x, 1, 2)  # [B, D, S]
        for b in range(B):
            out[:, b * SP + 1: b * SP + 1 + S] = xT[b]
        return np.ascontiguousarray(out)

    xq = pad_T(q)
    xk = pad_T(k)
    xv = np.ascontiguousarray(
        np.swapaxes(v, 1, 2).transpose(1, 0, 2).reshape(D, NPOS))

    def pack_w3(w_dev):  # [128co, 1024ci, 3t] -> [p, (t c), m] = [128,24,128]
        a = w_dev.transpose(1, 2, 0)          # [ci, t, co]
        a = a.reshape(8, 128, 3, 128)         # [c, p, t, co]
        return np.ascontiguousarray(
            a.transpose(1, 2, 0, 3).reshape(128, 24, 128))

    def pack_w1(w_dev):  # [128co, 1024ci] -> [p, c, m] = [128, 8, 128]
        a = w_dev.T.reshape(8, 128, 128)      # [c, p, co]
        return np.ascontiguousarray(a.transpose(1, 0, 2))

    cast = lambda a: a.astype(bf16)
    in_maps = []
    bias_y = np.zeros((D,), np.float32)
    for dev in range(NCORES):
        heads = [2 * dev, 2 * dev + 1]
        rows = np.array([di * H + h for h in heads for di in range(HD)])
        feat = slice(2 * dev * HD, 2 * dev * HD + 128)
        wc_slice = np.ascontiguousarray(wc_w[:, feat].T)   # [128, 1024]
        bv_dev = np.asarray(inputs["wv_b"], np.float32)[rows]
        bias_y += bv_dev @ wc_slice
        in_maps.append({
            "xq": cast(xq), "xk": cast(xk), "xv": cast(xv),
            "wq": cast(pack_w3(wq_w[rows])),
            "wk": cast(pack_w3(wk_w[rows])),
            "wv": cast(pack_w1(wv_w[rows, :, 0])),
            "wc": cast(wc_slice),
            "bq": np.ascontiguousarray(
                np.asarray(inputs["wq_b"], np.float32)[rows][:, None]),
            "bk": np.ascontiguousarray(
                np.asarray(inputs["wk_b"], np.float32)[rows][:, None]),
        })
    bias_y += np.asarray(inputs["wc_b"], np.float32)
    return in_maps, bias_y


class Runner:
    """Caches the compiled module + jitted SPMD callable (mirrors
    bass2jax.run_bass_via_pjrt, but reusable across calls)."""

    def __init__(self, repeat: int = 1):
        import jax
        from jax.sharding import Mesh, PartitionSpec
        from jax.experimental.shard_map import shard_map
        from concourse.bass2jax import (
            _bass_exec_p, install_neuronx_cc_hook, partition_id_tensor)

        self.jax = jax
        nc = build_module(repeat)
        self.nc = nc
        install_neuronx_cc_hook()
        assert nc.dbg_addr is None

        in_names, out_names, out_avals, zero_outs = [], [], [], []
        pname = nc.partition_id_tensor.name if nc.partition_id_tensor else None
        for alloc in nc.m.functions[0].allocations:
            if not isinstance(alloc, mybir.MemoryLocationSet):
                continue
            name = alloc.memorylocations[0].name
            if alloc.kind == "ExternalInput":
                if name != pname:
                    in_names.append(name)
            elif alloc.kind == "ExternalOutput":
                out_names.append(name)
                shape = tuple(alloc.tensor_shape)
                dt = mybir.dt.np(alloc.dtype)
                out_avals.append(jax.core.ShapedArray(shape, dt))
                zero_outs.append(np.zeros(shape, dt))
        self.in_names, self.out_names = in_names, out_names
        self.out_avals, self.zero_outs = out_avals, zero_outs
        n_params, n_outs = len(in_names), len(out_avals)
        all_names = in_names + out_names + ([pname] if pname else [])

        def _body(*args):
            operands = list(args)
            if pname:
                operands.append(partition_id_tensor())
            return tuple(_bass_exec_p.bind(
                *operands,
                out_avals=tuple(out_avals),
                in_names=tuple(all_names),
                out_names=tuple(out_names),
                lowering_input_output_aliases=(),
                sim_require_finite=True,
                sim_require_nnan=True,
                nc=nc))

        devices = jax.devices()[:NCORES]
        self.mesh = Mesh(np.asarray(devices), ("core",))
        self.sharded = jax.jit(
            shard_map(_body, mesh=self.mesh,
                      in_specs=(PartitionSpec("core"),) * (n_params + n_outs),
                      out_specs=(PartitionSpec("core"),) * n_outs,
                      check_rep=False),
            donate_argnums=tuple(range(n_params, n_params + n_outs)),
            keep_unused=True)

    def concat_inputs(self, in_maps):
        return [np.concatenate([np.asarray(m[n]) for m in in_maps], axis=0)
                for n in self.in_names]

    def concat_zeros(self):
        return [np.zeros((NCORES * z.shape[0], *z.shape[1:]), z.dtype)
                for z in self.zero_outs]

    def call(self, concat_in, concat_zero):
        """Returns device output arrays (not fetched)."""
        out = self.sharded(*concat_in, *concat_zero)
        self.jax.block_until_ready(out)
        return out

    def run(self, in_maps):
        out = self.call(self.concat_inputs(in_maps), self.concat_zeros())
        return [
            {n: np.asarray(out[i]).reshape(NCORES, *self.out_avals[i].shape)[c]
             for i, n in enumerate(self.out_names)}
            for c in range(NCORES)]


_CACHED = {}


def get_runner(repeat: int = 1) -> Runner:
    if repeat not in _CACHED:
        _CACHED[repeat] = Runner(repeat)
    return _CACHED[repeat]


def run(in_maps, repeat: int = 1):
    return get_runner(repeat).run(in_maps)


def kernel(**inputs) -> np.ndarray:
    in_maps, bias_y = host_prep(inputs)
    results = run(in_maps)
    y = np.zeros((NPOS, D), np.float64)
    for r in results:
        y += r["y"].astype(np.float64)
    y = y.astype(np.float32) + bias_y[None, :]
    return y.reshape(B, S, D)


if __name__ == "__main__":
    rng = np.random.default_rng(0)
    fake = {
        "q": rng.standard_normal((B, S, D)).astype(np.float32),
        "k": rng.standard_normal((B, S, D)).astype(np.float32),
        "v": rng.standard_normal((B, S, D)).astype(np.float32),
        "wq_w": (rng.standard_normal((D, D, 3)) / 32).astype(np.float32),
        "wq_b": np.zeros(D, np.float32),
        "wk_w": (rng.standard_normal((D, D, 3)) / 32).astype(np.float32),
        "wk_b": np.zeros(D, np.float32),
        "wv_w": (rng.standard_normal((D, D, 1)) / 32).astype(np.float32),
        "wv_b": np.zeros(D, np.float32),
        "wc_w": (rng.standard_normal((D, D)) / 32).astype(np.float32),
        "wc_b": np.zeros(D, np.float32),
    }
    out = kernel(**fake)
    print("kernel output", out.shape, out.dtype)


# revision 25
# speedup vs baseline: 75.3563x; 1.0688x over previous
"""ConvMultiHeadAttention Trainium2 kernel (8-core SPMD, batch+head sharded).

Module: conv1d(k=3,pad=1) Q/K proj, conv1d(k=1) V proj, 16-head attention
(head = channel%16), concat, linear out-proj.  B=2, S=2048, D=1024, d=64.

Sharding: each of the 8 cores owns 2 heads x both batches.  Conv weights are
row-sliced per core (128 output channels each, ordered [head0 d0..63,
head1 d0..63]); q/k/v inputs are replicated (conv contracts all 1024 input
channels).  Each core produces a y-partial [4096, 1024] = (its heads' attn
output) @ wc_slice^T; the host sums the 8 partials and adds the biases that
commute out (wc_b, and bv @ wc_slice^T since softmax weights sum to 1).

v2 design (all matmuls bf16 = 1 cycle/row on PE; v1 was fp32 = 4):
  conv q/k  -> qcT/kcT [128ch, 4096pos] bf16 in SBUF; bias added by DVE
              tensor_scalar during the psum->sbuf copy.
  conv v    -> V0/V1 [128key, 32chunk, 65] bf16, col 64 memset to 1.0: the
              ones column makes attn@V also accumulate the softmax
              denominator r = sum_k E[k,q] for free.
  scores^T  sp[128key, 2head, 512q] psum (2 banks, one accumulation group
              per bank); exp -> E bf16 [128,2,512] on ACT (scale=1/8).
  attn@V    out[q,d] orientation: lhsT = E chunk [128k,128q], rhs =
              V[128k, 65] -> psum [128q, 65]; accumulate 16 chunks; full
              128x128 PE utilization (v1 orientation wasted half).
  softmax   rinv = 1/psum[:,64] (DVE); normalize fused into the psum->sbuf
              copy via DVE tensor_scalar_mul with per-partition scalar.
  transpose [q,feat] -> [feat,q] via DMA XBAR (SBUF->SBUF, no PSUM, ~112ns).
  proj      yp[128q, 512od] = outT.T @ wc (bf16); DVE copy -> y_sb f32;
              one [128,1024] DMA per 128-position chunk.

Emission is software-pipelined: tile t's scores/exp loop interleaves
conv-q of tile t+1 (fine-grained, PE gap-filler while ACT drains exp) and
the full attention-B of tile t-1 (attn@V + normalize + transpose + proj),
so PE and ACT both stay ~busy.  PSUM budget (8 banks): scores 2x[128,2,512]
(4) + conv [128,512] (1) + at0/at1 [128,65] (2) + proj [128,512] (1).
"""

import sys
import numpy as np
from contextlib import ExitStack

sys.path.insert(0, "/opt/trn_rl_repo")

import concourse.bass as bass
import concourse.tile as tile
from concourse import bacc, mybir
from concourse.bass_interp import get_hw_module
from concourse import bass2jax

F32 = mybir.dt.float32
BF16 = mybir.dt.bfloat16

NCORES = 8
B, S, D = 2, 2048, 1024
H, HD = 16, 64          # heads, head dim
CO = 128                # conv output channels per core (2 heads x 64)
SP = S + 2              # padded positions per batch for k=3 conv
NPOS = B * S            # 4096
NCHUNK = NPOS // 128    # 32 key chunks


def build_module(repeat: int = 1):
    nc = bacc.Bacc("TRN2", target_bir_lowering=False, debug=False,
                   num_devices=NCORES)

    xq = nc.dram_tensor("xq", [D, B * SP], BF16, kind="ExternalInput").ap()
    xk = nc.dram_tensor("xk", [D, B * SP], BF16, kind="ExternalInput").ap()
    xv = nc.dram_tensor("xv", [D, NPOS], BF16, kind="ExternalInput").ap()
    wq = nc.dram_tensor("wq", [128, 24, 128], BF16, kind="ExternalInput").ap()
    wk = nc.dram_tensor("wk", [128, 24, 128], BF16, kind="ExternalInput").ap()
    wv = nc.dram_tensor("wv", [128, 8, 128], BF16, kind="ExternalInput").ap()
    wc = nc.dram_tensor("wc", [128, 1024], BF16, kind="ExternalInput").ap()
    bq = nc.dram_tensor("bq", [128, 1], F32, kind="ExternalInput").ap()
    bk = nc.dram_tensor("bk", [128, 1], F32, kind="ExternalInput").ap()
    ident = nc.dram_tensor("ident", [128, 128], BF16,
                           kind="ExternalInput").ap()
    y = nc.dram_tensor("y", [NPOS, D], BF16, kind="ExternalOutput").ap()

    with tile.TileContext(nc) as tc, ExitStack() as ctx:
        wpool = ctx.enter_context(tc.tile_pool(name="wpool", bufs=1))
        cpool = ctx.enter_context(tc.tile_pool(name="cpool", bufs=1))
        xpool = ctx.enter_context(tc.tile_pool(name="xpool", bufs=3))
        epool = ctx.enter_context(tc.tile_pool(name="epool", bufs=34))
        spool = ctx.enter_context(tc.tile_pool(name="spool", bufs=2))
        ypool = ctx.enter_context(tc.tile_pool(name="ypool", bufs=3))

        # ---- persistent weights (ACT hwdge queue, so the first x-tile
        # loads on the SP queue aren't stuck behind them) ----
        wq_sb = wpool.tile([128, 24, 128], BF16)
        nc.scalar.dma_start(wq_sb[:], wq[:])
        wk_sb = wpool.tile([128, 24, 128], BF16)
        wv_sb = wpool.tile([128, 8, 128], BF16)
        wc_sb = wpool.tile([128, 1024], BF16)
        bq_sb = wpool.tile([128, 1], F32)
        bk_sb = wpool.tile([128, 1], F32)
        nc.scalar.dma_start(wk_sb[:], wk[:])
        nc.scalar.dma_start(wv_sb[:], wv[:])
        nc.scalar.dma_start(wc_sb[:], wc[:])
        nc.scalar.dma_start(bq_sb[:], bq[:])
        nc.scalar.dma_start(bk_sb[:], bk[:])
        ident_sb = wpool.tile([128, 128], BF16)
        nc.scalar.dma_start(ident_sb[:], ident[:])

        # ---- persistent activations ----
        qcT = cpool.tile([128, NPOS], BF16)
        kcT = cpool.tile([128, NPOS], BF16)
        V0 = cpool.tile([128, NCHUNK, 65], BF16)   # head0 V + ones col
        V1 = cpool.tile([128, NCHUNK, 65], BF16)   # head1 V + ones col
        nc.vector.memset(V0[:, :, 64:65], 1.0)
        nc.vector.memset(V1[:, :, 64:65], 1.0)

        # PSUM pools: sp 2x[128,2,512] (4 banks) + cq [128,512] (1) +
        # at0/at1 [128,65] (2) + yp [128,512] (1) = 8 banks exactly.
        psp = ctx.enter_context(tc.tile_pool(name="psp", bufs=2, space="PSUM"))
        pcq = ctx.enter_context(tc.tile_pool(name="pcq", bufs=1, space="PSUM"))
        pat = ctx.enter_context(tc.tile_pool(name="pat", bufs=1, space="PSUM"))
        pyp = ctx.enter_context(tc.tile_pool(name="pyp", bufs=1, space="PSUM"))

        def load_xqk_pair(src, b, jp):
            """One 1026-wide load serving two adjacent 512-col conv tiles."""
            xt = xpool.tile([128, 8, 1026], BF16, tag="xqk", bufs=2)
            col0 = b * SP + jp * 1024
            nc.sync.dma_start(
                xt[:],
                src[:, col0:col0 + 1026].rearrange("(c p) i -> p c i", p=128))
            return xt

        def conv_qk_mms(xt, joff, w_sb):
            """Yields the 24 accumulating matmul thunks for one 512-col tile
            (slice joff in {0,1} of a pair-load); the caller paces them."""
            ps = pcq.tile([128, 512], F32, tag="cq")
            thunks = []
            for t in range(3):
                for c in range(8):
                    n = t * 8 + c
                    o = joff * 512
                    def mm(n=n, t=t, c=c, o=o, ps=ps, xt=xt, w_sb=w_sb):
                        nc.tensor.matmul(ps[:], w_sb[:, n, :],
                                         xt[:, c, o + t:o + t + 512],
                                         start=(n == 0), stop=(n == 23))
                    thunks.append(mm)
            return ps, thunks

        def conv_qk_copy(outT, ps, b_sb, b, j):
            cols = slice(b * S + j * 512, b * S + (j + 1) * 512)
            nc.vector.tensor_scalar_add(outT[:, cols], ps[:], b_sb[:, 0:1])

        def conv_v_unit(b, j):
            """One 512-position V tile: load, 32 matmuls, 2 copies."""
            xt = xpool.tile([128, 8, 512], BF16, tag="xv", bufs=2)
            col0 = b * S + j * 512
            nc.sync.dma_start(
                xt[:],
                xv[:, col0:col0 + 512].rearrange("(c p) i -> p c i", p=128))
            vp = pcq.tile([128, 4, 128], F32, tag="cq")
            for g in range(4):
                for c in range(8):
                    nc.tensor.matmul(vp[:, g, :], xt[:, c, g * 128:(g + 1) * 128],
                                     wv_sb[:, c, :],
                                     start=(c == 0), stop=(c == 7))
            c0 = b * 16 + j * 4
            nc.vector.tensor_copy(V0[:, c0:c0 + 4, 0:64], vp[:, :, 0:64])
            nc.vector.tensor_copy(V1[:, c0:c0 + 4, 0:64], vp[:, :, 64:128])

        def scores_chunk(b, jq, c, e_tiles):
            """Scores + exp for key-chunk c of tile (b, jq)."""
            q0 = b * S + jq * 512
            k0 = b * S + c * 128
            sp = psp.tile([128, 2, 512], F32, tag="sp")
            nc.tensor.matmul(sp[:, 0, :], kcT[0:64, k0:k0 + 128],
                             qcT[0:64, q0:q0 + 512], start=True, stop=True)
            nc.tensor.matmul(sp[:, 1, :], kcT[64:128, k0:k0 + 128],
                             qcT[64:128, q0:q0 + 512], start=True, stop=True)
            e = epool.tile([128, 2, 512], BF16, tag="e")
            nc.scalar.activation(e[:], sp[:],
                                 mybir.ActivationFunctionType.Exp, scale=0.125)
            e_tiles.append(e)

        def attn_b_stream(b, jq, e_tiles):
            """Yields work units for the B-phase of tile (b, jq): per qgroup
            32 attn@V matmul thunks, a normalize+transpose thunk, and —
            lagged one qgroup so attn@V matmuls hide the single-bank proj
            ping-pong — a proj+store thunk."""
            outT_by_qg = {}
            y_tile_box = [None]
            pending_proj = None
            for qg in range(4):
                at0 = pat.tile([128, 65], F32, tag="at0")
                at1 = pat.tile([128, 65], F32, tag="at1")
                qs = slice(qg * 128, (qg + 1) * 128)
                for c in range(16):
                    cb = b * 16 + c
                    def mm(c=c, cb=cb, at0=at0, at1=at1, qs=qs):
                        e = e_tiles[c]
                        nc.tensor.matmul(at0[:], e[:, 0, qs], V0[:, cb, :],
                                         start=(c == 0), stop=(c == 15))
                        nc.tensor.matmul(at1[:], e[:, 1, qs], V1[:, cb, :],
                                         start=(c == 0), stop=(c == 15))
                    yield ("mm", mm)
                    if c == 7 and pending_proj is not None:
                        yield ("finish", pending_proj)
                        pending_proj = None

                def norm_tr(qg=qg, at0=at0, at1=at1):
                    norm = spool.tile([128, 128], BF16, tag="norm", bufs=2)
                    outT = spool.tile([128, 128], BF16, tag="outT", bufs=3)
                    for h, at in ((0, at0), (1, at1)):
                        rinv = spool.tile([128, 1], F32, tag="rinv", bufs=4)
                        nc.vector.reciprocal(rinv[:], at[:, 64:65])
                        nc.vector.tensor_scalar_mul(
                            norm[:, h * 64:(h + 1) * 64],
                            at[:, 0:64], rinv[:, 0:1])
                    # [q, feat] -> [feat, q]: PE transpose through the proj
                    # psum slot (DMA-XBAR transposes cost ~2.6us each in
                    # per-op DMA overhead on this fabric)
                    trp = pyp.tile([128, 128], BF16, tag="yp")
                    nc.tensor.transpose(trp[:], norm[:], ident_sb[:])
                    nc.vector.tensor_copy(outT[:], trp[:])
                    outT_by_qg[qg] = outT
                yield ("finish", norm_tr)

                def proj(qg=qg, b=b, jq=jq):
                    outT = outT_by_qg.pop(qg)
                    if qg == 0:
                        y_new = ypool.tile([128, 4, 1024], BF16,
                                           tag="ysb", bufs=2)
                        y_tile_box[0] = y_new
                    y_sb = y_tile_box[0]
                    for half in range(2):
                        yp = pyp.tile([128, 512], F32, tag="yp")
                        nc.tensor.matmul(
                            yp[:], outT[:],
                            wc_sb[:, half * 512:(half + 1) * 512],
                            start=True, stop=True)
                        nc.vector.tensor_copy(
                            y_sb[:, qg, half * 512:(half + 1) * 512], yp[:])
                    if qg == 3:
                        row0 = b * S + jq * 512
                        nc.sync.dma_start(
                            y[row0:row0 + 512, :].rearrange(
                                "(j p) d -> p j d", p=128), y_sb[:])
                pending_proj = proj
            yield ("finish", pending_proj)

        def body():
            # ---------- prologue: conv q(0,0)+(0,1), conv k b0 + scores t0 --
            xq_pair = load_xqk_pair(xq, 0, 0)   # serves conv-q of tiles 0,1
            ps, thunks = conv_qk_mms(xq_pair, 0, wq_sb)
            for mm in thunks:
                mm()
            conv_qk_copy(qcT, ps, bq_sb, 0, 0)
            e_t0 = []
            for jp in range(2):
                xt = load_xqk_pair(xk, 0, jp)
                for joff in range(2):
                    j = jp * 2 + joff
                    ps, thunks = conv_qk_mms(xt, joff, wk_sb)
                    for mm in thunks:
                        mm()
                    conv_qk_copy(kcT, ps, bk_sb, 0, j)
                    for c in range(4 * j, 4 * j + 4):
                        scores_chunk(0, 0, c, e_t0)
            for j in range(4):
                conv_v_unit(0, j)
            # conv q for tile 1 (b0, jq1) — tile t's conv-q runs in tile t-1
            ps, thunks = conv_qk_mms(xq_pair, 1, wq_sb)
            for mm in thunks:
                mm()
            conv_qk_copy(qcT, ps, bq_sb, 0, 1)
            e_prev = e_t0

            # deferred b1 conv work, drained 2 units/tile across t1..t3
            def ck(jp):
                xt = load_xqk_pair(xk, 1, jp)
                for joff in range(2):
                    ps, thunks = conv_qk_mms(xt, joff, wk_sb)
                    for mm in thunks:
                        mm()
                    conv_qk_copy(kcT, ps, bk_sb, 1, jp * 2 + joff)
            b1_work = [lambda: ck(0), lambda: conv_v_unit(1, 0),
                       lambda: ck(1), lambda: conv_v_unit(1, 1),
                       lambda: conv_v_unit(1, 2), lambda: conv_v_unit(1, 3)]
            b1_per_tile = 2

            # ---------- steady tiles t = 1..7 + epilogue ----------
            for t in range(1, 9):
                # staggered-reset stage boundaries: stages = {pro+t1},
                # {t2,t3}, {t4,t5}, {t6,t7,epi}.  b0 stages touch only
                # b0 slices of qcT/kcT/V/y (and vice versa), so adjacent-
                # stage overlap across the back edge is data-disjoint.
                if staggered and t in (2, 4, 6):
                    tc.stage_boundary()
                b, jq = divmod(t, 4) if t < 8 else (None, None)
                e_cur = []
                bstream = attn_b_stream((t - 1) // 4, (t - 1) % 4, e_prev)

                if t < 8:
                    # conv-q(t) ran during tile t-1; here interleave
                    # conv-q(t+1) into the scores/exp loop as PE gap-filler.
                    nb, njq = divmod(t + 1, 4) if t + 1 < 8 else (None, None)
                    cq_thunks = []
                    cq_ps = None
                    if nb is not None:
                        if njq % 2 == 0:
                            xq_pair = load_xqk_pair(xq, nb, njq // 2)
                        cq_ps, cq_thunks = conv_qk_mms(xq_pair, njq % 2,
                                                       wq_sb)

                    # interleave: 16 chunk-steps
                    cqi = 0
                    for c in range(16):
                        scores_chunk(b, jq, c, e_cur)
                        # pace conv-q: 24 mms over 16 steps
                        target = (c + 1) * len(cq_thunks) // 16
                        while cqi < target:
                            cq_thunks[cqi]()
                            cqi += 1
                        # pace attn-B of t-1: 64 mm-units over 16 steps
                        units = 0
                        for kind, fn in bstream:
                            fn()
                            if kind == "mm":
                                units += 1
                                if units >= 4:
                                    break
                            # finish units don't count against the pace
                    # drain remaining B-stream units
                    for kind, fn in bstream:
                        fn()
                    if cq_ps is not None:
                        conv_qk_copy(qcT, cq_ps, bq_sb, nb, njq)
                    # b1 conv work during t1..t3
                    if t <= 3:
                        for _ in range(b1_per_tile):
                            if b1_work:
                                b1_work.pop(0)()
                    e_prev = e_cur
                else:
                    # epilogue: drain B of t7
                    for kind, fn in bstream:
                        fn()

        if repeat == 1:
            staggered = False
            body()
        else:
            staggered = True
            with tc.For_i(0, repeat, 1, staggered_reset=True,
                          hint_engines=(mybir.EngineType.PE,
                                        mybir.EngineType.Activation,
                                        mybir.EngineType.DVE,
                                        mybir.EngineType.SP)):
                body()

    nc.compile()
    nc.m = get_hw_module(nc.m)
    return nc


def host_prep(inputs):
    """Returns (in_maps, bias_y) — per-core input dicts + host-side bias."""
    import ml_dtypes
    bf16 = ml_dtypes.bfloat16
    q = np.asarray(inputs["q"], np.float32)
    k = np.asarray(inputs["k"], np.float32)
    v = np.asarray(inputs["v"], np.float32)
    wq_w = np.asarray(inputs["wq_w"], np.float32)
    wk_w = np.asarray(inputs["wk_w"], np.float32)
    wv_w = np.asarray(inputs["wv_w"], np.float32)
    wc_w = np.asarray(inputs["wc_w"], np.float32)

    def pad_T(x):  # [B,S,D] -> [D, B*(S+2)] zero-padded at batch edges
        out = np.zeros((D, B * SP), np.float32)
        xT = np.swapaxes(x, 1, 2)  # [B, D, S]
        for b in range(B):
            out[:, b * SP + 1: b * SP + 1 + S] = xT[b]
        return np.ascontiguousarray(out)

    xq = pad_T(q)
    xk = pad_T(k)
    xv = np.ascontiguousarray(
        np.swapaxes(v, 1, 2).transpose(1, 0, 2).reshape(D, NPOS))

    def pack_w3(w_dev):  # [128co, 1024ci, 3t] -> [p, (t c), m] = [128,24,128]
        a = w_dev.transpose(1, 2, 0)          # [ci, t, co]
        a = a.reshape(8, 128, 3, 128)         # [c, p, t, co]
        return np.ascontiguousarray(
            a.transpose(1, 2, 0, 3).reshape(128, 24, 128))

    def pack_w1(w_dev):  # [128co, 1024ci] -> [p, c, m] = [128, 8, 128]
        a = w_dev.T.reshape(8, 128, 128)      # [c, p, co]
        return np.ascontiguousarray(a.transpose(1, 0, 2))

    cast = lambda a: a.astype(bf16)
    in_maps = []
    bias_y = np.zeros((D,), np.float32)
    for dev in range(NCORES):
        heads = [2 * dev, 2 * dev + 1]
        rows = np.array([di * H + h for h in heads for di in range(HD)])
        feat = slice(2 * dev * HD, 2 * dev * HD + 128)
        wc_slice = np.ascontiguousarray(wc_w[:, feat].T)   # [128, 1024]
        bv_dev = np.asarray(inputs["wv_b"], np.float32)[rows]
        bias_y += bv_dev @ wc_slice
        in_maps.append({
            "xq": cast(xq), "xk": cast(xk), "xv": cast(xv),
            "ident": np.eye(128, dtype=bf16),
            "wq": cast(pack_w3(wq_w[rows])),
            "wk": cast(pack_w3(wk_w[rows])),
            "wv": cast(pack_w1(wv_w[rows, :, 0])),
            "wc": cast(wc_slice),
            "bq": np.ascontiguousarray(
                np.asarray(inputs["wq_b"], np.float32)[rows][:, None]),
            "bk": np.ascontiguousarray(
                np.asarray(inputs["wk_b"], np.float32)[rows][:, None]),
        })
    bias_y += np.asarray(inputs["wc_b"], np.float32)
    return in_maps, bias_y


class Runner:
    """Caches the compiled module + jitted SPMD callable (mirrors
    bass2jax.run_bass_via_pjrt, but reusable across calls)."""

    def __init__(self, repeat: int = 1):
        import jax
        from jax.sharding import Mesh, PartitionSpec
        from jax.experimental.shard_map import shard_map
        from concourse.bass2jax import (
            _bass_exec_p, install_neuronx_cc_hook, partition_id_tensor)

        self.jax = jax
        nc = build_module(repeat)
        self.nc = nc
        install_neuronx_cc_hook()
        assert nc.dbg_addr is None

        in_names, out_names, out_avals, zero_outs = [], [], [], []
        pname = nc.partition_id_tensor.name if nc.partition_id_tensor else None
        for alloc in nc.m.functions[0].allocations:
            if not isinstance(alloc, mybir.MemoryLocationSet):
                continue
            name = alloc.memorylocations[0].name
            if alloc.kind == "ExternalInput":
                if name != pname:
                    in_names.append(name)
            elif alloc.kind == "ExternalOutput":
                out_names.append(name)
                shape = tuple(alloc.tensor_shape)
                dt = mybir.dt.np(alloc.dtype)
                out_avals.append(jax.core.ShapedArray(shape, dt))
                zero_outs.append(np.zeros(shape, dt))
        self.in_names, self.out_names = in_names, out_names
        self.out_avals, self.zero_outs = out_avals, zero_outs
        n_params, n_outs = len(in_names), len(out_avals)
        all_names = in_names + out_names + ([pname] if pname else [])

        def _body(*args):
            operands = list(args)
            if pname:
                operands.append(partition_id_tensor())
            return tuple(_bass_exec_p.bind(
                *operands,
                out_avals=tuple(out_avals),
                in_names=tuple(all_names),
                out_names=tuple(out_names),
                lowering_input_output_aliases=(),
                sim_require_finite=True,
                sim_require_nnan=True,
                nc=nc))

        devices = jax.devices()[:NCORES]
        self.mesh = Mesh(np.asarray(devices), ("core",))
        self.sharded = jax.jit(
            shard_map(_body, mesh=self.mesh,
                      in_specs=(PartitionSpec("core"),) * (n_params + n_outs),
                      out_specs=(PartitionSpec("core"),) * n_outs,
                      check_rep=False),
            donate_argnums=tuple(range(n_params, n_params + n_outs)),
            keep_unused=True)

    def concat_inputs(self, in_maps):
        return [np.concatenate([np.asarray(m[n]) for m in in_maps], axis=0)
                for n in self.in_names]

    def concat_zeros(self):
        return [np.zeros((NCORES * z.shape[0], *z.shape[1:]), z.dtype)
                for z in self.zero_outs]

    def call(self, concat_in, concat_zero):
        """Returns device output arrays (not fetched)."""
        out = self.sharded(*concat_in, *concat_zero)
        self.jax.block_until_ready(out)
        return out

    def run(self, in_maps):
        out = self.call(self.concat_inputs(in_maps), self.concat_zeros())
        return [
            {n: np.asarray(out[i]).reshape(NCORES, *self.out_avals[i].shape)[c]
             for i, n in enumerate(self.out_names)}
            for c in range(NCORES)]


_CACHED = {}


def get_runner(repeat: int = 1) -> Runner:
    if repeat not in _CACHED:
        _CACHED[repeat] = Runner(repeat)
    return _CACHED[repeat]


def run(in_maps, repeat: int = 1):
    return get_runner(repeat).run(in_maps)


def kernel(**inputs) -> np.ndarray:
    in_maps, bias_y = host_prep(inputs)
    results = run(in_maps)
    y = np.zeros((NPOS, D), np.float64)
    for r in results:
        y += r["y"].astype(np.float64)
    y = y.astype(np.float32) + bias_y[None, :]
    return y.reshape(B, S, D)


if __name__ == "__main__":
    rng = np.random.default_rng(0)
    fake = {
        "q": rng.standard_normal((B, S, D)).astype(np.float32),
        "k": rng.standard_normal((B, S, D)).astype(np.float32),
        "v": rng.standard_normal((B, S, D)).astype(np.float32),
        "wq_w": (rng.standard_normal((D, D, 3)) / 32).astype(np.float32),
        "wq_b": np.zeros(D, np.float32),
        "wk_w": (rng.standard_normal((D, D, 3)) / 32).astype(np.float32),
        "wk_b": np.zeros(D, np.float32),
        "wv_w": (rng.standard_normal((D, D, 1)) / 32).astype(np.float32),
        "wv_b": np.zeros(D, np.float32),
        "wc_w": (rng.standard_normal((D, D)) / 32).astype(np.float32),
        "wc_b": np.zeros(D, np.float32),
    }
    out = kernel(**fake)
    print("kernel output", out.shape, out.dtype)


# revision 26
# speedup vs baseline: 80.0110x; 1.0618x over previous
"""ConvMultiHeadAttention Trainium2 kernel (8-core SPMD, batch+head sharded).

Module: conv1d(k=3,pad=1) Q/K proj, conv1d(k=1) V proj, 16-head attention
(head = channel%16), concat, linear out-proj.  B=2, S=2048, D=1024, d=64.

Sharding: each of the 8 cores owns 2 heads x both batches.  Conv weights are
row-sliced per core (128 output channels each, ordered [head0 d0..63,
head1 d0..63]); q/k/v inputs are replicated (conv contracts all 1024 input
channels).  Each core produces a y-partial [4096, 1024] = (its heads' attn
output) @ wc_slice^T; the host sums the 8 partials and adds the biases that
commute out (wc_b, and bv @ wc_slice^T since softmax weights sum to 1).

v2 design (all matmuls bf16 = 1 cycle/row on PE; v1 was fp32 = 4):
  conv q/k  -> qcT/kcT [128ch, 4096pos] bf16 in SBUF; bias added by DVE
              tensor_scalar during the psum->sbuf copy.
  conv v    -> V0/V1 [128key, 32chunk, 65] bf16, col 64 memset to 1.0: the
              ones column makes attn@V also accumulate the softmax
              denominator r = sum_k E[k,q] for free.
  scores^T  sp[128key, 2head, 512q] psum (2 banks, one accumulation group
              per bank); exp -> E bf16 [128,2,512] on ACT (scale=1/8).
  attn@V    out[q,d] orientation: lhsT = E chunk [128k,128q], rhs =
              V[128k, 65] -> psum [128q, 65]; accumulate 16 chunks; full
              128x128 PE utilization (v1 orientation wasted half).
  softmax   rinv = 1/psum[:,64] (DVE); normalize fused into the psum->sbuf
              copy via DVE tensor_scalar_mul with per-partition scalar.
  transpose [q,feat] -> [feat,q] via DMA XBAR (SBUF->SBUF, no PSUM, ~112ns).
  proj      yp[128q, 512od] = outT.T @ wc (bf16); DVE copy -> y_sb f32;
              one [128,1024] DMA per 128-position chunk.

Emission is software-pipelined: tile t's scores/exp loop interleaves
conv-q of tile t+1 (fine-grained, PE gap-filler while ACT drains exp) and
the full attention-B of tile t-1 (attn@V + normalize + transpose + proj),
so PE and ACT both stay ~busy.  PSUM budget (8 banks): scores 2x[128,2,512]
(4) + conv [128,512] (1) + at0/at1 [128,65] (2) + proj [128,512] (1).
"""

import sys
import numpy as np
from contextlib import ExitStack

sys.path.insert(0, "/opt/trn_rl_repo")

import concourse.bass as bass
import concourse.tile as tile
from concourse import bacc, mybir
from concourse.bass_interp import get_hw_module
from concourse import bass2jax

F32 = mybir.dt.float32
BF16 = mybir.dt.bfloat16

NCORES = 8
B, S, D = 2, 2048, 1024
H, HD = 16, 64          # heads, head dim
CO = 128                # conv output channels per core (2 heads x 64)
SP = S + 2              # padded positions per batch for k=3 conv
NPOS = B * S            # 4096
NCHUNK = NPOS // 128    # 32 key chunks


def build_module(repeat: int = 1):
    nc = bacc.Bacc("TRN2", target_bir_lowering=False, debug=False,
                   num_devices=NCORES)

    xq = nc.dram_tensor("xq", [D, B * SP], BF16, kind="ExternalInput").ap()
    xk = nc.dram_tensor("xk", [D, B * SP], BF16, kind="ExternalInput").ap()
    xv = nc.dram_tensor("xv", [D, NPOS], BF16, kind="ExternalInput").ap()
    wq = nc.dram_tensor("wq", [128, 24, 128], BF16, kind="ExternalInput").ap()
    wk = nc.dram_tensor("wk", [128, 24, 128], BF16, kind="ExternalInput").ap()
    wv = nc.dram_tensor("wv", [128, 8, 128], BF16, kind="ExternalInput").ap()
    wc = nc.dram_tensor("wc", [128, 1024], BF16, kind="ExternalInput").ap()
    bq = nc.dram_tensor("bq", [128, 1], F32, kind="ExternalInput").ap()
    bk = nc.dram_tensor("bk", [128, 1], F32, kind="ExternalInput").ap()
    ident = nc.dram_tensor("ident", [128, 128], BF16,
                           kind="ExternalInput").ap()
    y = nc.dram_tensor("y", [NPOS, D], BF16, kind="ExternalOutput").ap()

    with tile.TileContext(nc) as tc, ExitStack() as ctx:
        wpool = ctx.enter_context(tc.tile_pool(name="wpool", bufs=1))
        cpool = ctx.enter_context(tc.tile_pool(name="cpool", bufs=1))
        xpool = ctx.enter_context(tc.tile_pool(name="xpool", bufs=3))
        epool = ctx.enter_context(tc.tile_pool(name="epool", bufs=34))
        spool = ctx.enter_context(tc.tile_pool(name="spool", bufs=2))
        ypool = ctx.enter_context(tc.tile_pool(name="ypool", bufs=3))

        # ---- persistent weights (ACT hwdge queue, so the first x-tile
        # loads on the SP queue aren't stuck behind them) ----
        wq_sb = wpool.tile([128, 24, 128], BF16)
        nc.scalar.dma_start(wq_sb[:], wq[:])
        wk_sb = wpool.tile([128, 24, 128], BF16)
        wv_sb = wpool.tile([128, 8, 128], BF16)
        wc_sb = wpool.tile([128, 1024], BF16)
        bq_sb = wpool.tile([128, 1], F32)
        bk_sb = wpool.tile([128, 1], F32)
        nc.scalar.dma_start(wk_sb[:], wk[:])
        nc.scalar.dma_start(wv_sb[:], wv[:])
        nc.scalar.dma_start(wc_sb[:], wc[:])
        nc.scalar.dma_start(bq_sb[:], bq[:])
        nc.scalar.dma_start(bk_sb[:], bk[:])
        ident_sb = wpool.tile([128, 128], BF16)
        nc.scalar.dma_start(ident_sb[:], ident[:])

        # ---- persistent activations ----
        qcT = cpool.tile([128, NPOS], BF16)
        kcT = cpool.tile([128, NPOS], BF16)
        V0 = cpool.tile([128, NCHUNK, 65], BF16)   # head0 V + ones col
        V1 = cpool.tile([128, NCHUNK, 65], BF16)   # head1 V + ones col
        nc.vector.memset(V0[:, :, 64:65], 1.0)
        nc.vector.memset(V1[:, :, 64:65], 1.0)

        # PSUM pools: sp 2x[128,2,512] (4 banks) + cq [128,512] (1) +
        # at0/at1 [128,65] (2) + yp [128,512] (1) = 8 banks exactly.
        psp = ctx.enter_context(tc.tile_pool(name="psp", bufs=2, space="PSUM"))
        pcq = ctx.enter_context(tc.tile_pool(name="pcq", bufs=1, space="PSUM"))
        pat = ctx.enter_context(tc.tile_pool(name="pat", bufs=1, space="PSUM"))
        pyp = ctx.enter_context(tc.tile_pool(name="pyp", bufs=1, space="PSUM"))

        def load_xqk_pair(src, b, jp):
            """One 1026-wide load serving two adjacent 512-col conv tiles."""
            xt = xpool.tile([128, 8, 1026], BF16, tag="xqk", bufs=2)
            col0 = b * SP + jp * 1024
            nc.sync.dma_start(
                xt[:],
                src[:, col0:col0 + 1026].rearrange("(c p) i -> p c i", p=128))
            return xt

        def conv_qk_mms(xt, joff, w_sb):
            """Yields the 24 accumulating matmul thunks for one 512-col tile
            (slice joff in {0,1} of a pair-load); the caller paces them."""
            ps = pcq.tile([128, 512], F32, tag="cq")
            thunks = []
            for t in range(3):
                for c in range(8):
                    n = t * 8 + c
                    o = joff * 512
                    def mm(n=n, t=t, c=c, o=o, ps=ps, xt=xt, w_sb=w_sb):
                        nc.tensor.matmul(ps[:], w_sb[:, n, :],
                                         xt[:, c, o + t:o + t + 512],
                                         start=(n == 0), stop=(n == 23))
                    thunks.append(mm)
            return ps, thunks

        def conv_qk_copy(outT, ps, b_sb, b, j):
            cols = slice(b * S + j * 512, b * S + (j + 1) * 512)
            nc.vector.tensor_scalar_add(outT[:, cols], ps[:], b_sb[:, 0:1])

        def conv_v_unit(b, j):
            """One 512-position V tile: load, 32 matmuls, 2 copies."""
            xt = xpool.tile([128, 8, 512], BF16, tag="xv", bufs=2)
            col0 = b * S + j * 512
            nc.sync.dma_start(
                xt[:],
                xv[:, col0:col0 + 512].rearrange("(c p) i -> p c i", p=128))
            vp = pcq.tile([128, 4, 128], F32, tag="cq")
            for g in range(4):
                for c in range(8):
                    nc.tensor.matmul(vp[:, g, :], xt[:, c, g * 128:(g + 1) * 128],
                                     wv_sb[:, c, :],
                                     start=(c == 0), stop=(c == 7))
            c0 = b * 16 + j * 4
            nc.vector.tensor_copy(V0[:, c0:c0 + 4, 0:64], vp[:, :, 0:64])
            nc.vector.tensor_copy(V1[:, c0:c0 + 4, 0:64], vp[:, :, 64:128])

        def scores_chunk(b, jq, c, e_tiles):
            """Scores + exp for key-chunk c of tile (b, jq)."""
            q0 = b * S + jq * 512
            k0 = b * S + c * 128
            sp = psp.tile([128, 2, 512], F32, tag="sp")
            nc.tensor.matmul(sp[:, 0, :], kcT[0:64, k0:k0 + 128],
                             qcT[0:64, q0:q0 + 512], start=True, stop=True)
            nc.tensor.matmul(sp[:, 1, :], kcT[64:128, k0:k0 + 128],
                             qcT[64:128, q0:q0 + 512], start=True, stop=True)
            e = epool.tile([128, 2, 512], BF16, tag="e")
            nc.scalar.activation(e[:], sp[:],
                                 mybir.ActivationFunctionType.Exp, scale=0.125)
            e_tiles.append(e)

        def attn_b_stream(b, jq, e_tiles):
            """Yields work units for the B-phase of tile (b, jq): per qgroup
            32 attn@V matmul thunks, a normalize+transpose thunk, and —
            lagged one qgroup so attn@V matmuls hide the single-bank proj
            ping-pong — a proj+store thunk."""
            outT_by_qg = {}
            y_tile_box = [None]
            pending_proj = None
            for qg in range(4):
                at0 = pat.tile([128, 65], F32, tag="at0")
                at1 = pat.tile([128, 65], F32, tag="at1")
                qs = slice(qg * 128, (qg + 1) * 128)
                for c in range(16):
                    cb = b * 16 + c
                    def mm(c=c, cb=cb, at0=at0, at1=at1, qs=qs):
                        e = e_tiles[c]
                        nc.tensor.matmul(at0[:], e[:, 0, qs], V0[:, cb, :],
                                         start=(c == 0), stop=(c == 15))
                        nc.tensor.matmul(at1[:], e[:, 1, qs], V1[:, cb, :],
                                         start=(c == 0), stop=(c == 15))
                    yield ("mm", mm)
                    if c == 7 and pending_proj is not None:
                        yield ("finish", pending_proj)
                        pending_proj = None

                def norm_tr(qg=qg, at0=at0, at1=at1):
                    norm = spool.tile([128, 128], BF16, tag="norm", bufs=2)
                    outT = spool.tile([128, 128], BF16, tag="outT", bufs=3)
                    for h, at in ((0, at0), (1, at1)):
                        rinv = spool.tile([128, 1], F32, tag="rinv", bufs=4)
                        nc.vector.reciprocal(rinv[:], at[:, 64:65])
                        nc.vector.tensor_scalar_mul(
                            norm[:, h * 64:(h + 1) * 64],
                            at[:, 0:64], rinv[:, 0:1])
                    # [q, feat] -> [feat, q]: PE transpose through the proj
                    # psum slot (DMA-XBAR transposes cost ~2.6us each in
                    # per-op DMA overhead on this fabric)
                    trp = pyp.tile([128, 128], BF16, tag="yp")
                    nc.tensor.transpose(trp[:], norm[:], ident_sb[:])
                    nc.vector.tensor_copy(outT[:], trp[:])
                    outT_by_qg[qg] = outT
                yield ("finish", norm_tr)

                def proj(qg=qg, b=b, jq=jq):
                    outT = outT_by_qg.pop(qg)
                    if qg == 0:
                        y_new = ypool.tile([128, 4, 1024], BF16,
                                           tag="ysb", bufs=2)
                        y_tile_box[0] = y_new
                    y_sb = y_tile_box[0]
                    for half in range(2):
                        yp = pyp.tile([128, 512], F32, tag="yp")
                        nc.tensor.matmul(
                            yp[:], outT[:],
                            wc_sb[:, half * 512:(half + 1) * 512],
                            start=True, stop=True)
                        nc.vector.tensor_copy(
                            y_sb[:, qg, half * 512:(half + 1) * 512], yp[:])
                    if qg == 3:
                        row0 = b * S + jq * 512
                        nc.sync.dma_start(
                            y[row0:row0 + 512, :].rearrange(
                                "(j p) d -> p j d", p=128), y_sb[:])
                pending_proj = proj
            yield ("finish", pending_proj)

        def body():
            # ---------- prologue: conv q(0,0)+(0,1), conv k b0 + scores t0 --
            xq_pair = load_xqk_pair(xq, 0, 0)   # serves conv-q of tiles 0,1
            ps, thunks = conv_qk_mms(xq_pair, 0, wq_sb)
            for mm in thunks:
                mm()
            conv_qk_copy(qcT, ps, bq_sb, 0, 0)
            e_t0 = []
            for jp in range(2):
                xt = load_xqk_pair(xk, 0, jp)
                for joff in range(2):
                    j = jp * 2 + joff
                    ps, thunks = conv_qk_mms(xt, joff, wk_sb)
                    for mm in thunks:
                        mm()
                    conv_qk_copy(kcT, ps, bk_sb, 0, j)
                    for c in range(4 * j, 4 * j + 4):
                        scores_chunk(0, 0, c, e_t0)
            for j in range(4):
                conv_v_unit(0, j)
            # conv q for tile 1 (b0, jq1) — tile t's conv-q runs in tile t-1
            ps, thunks = conv_qk_mms(xq_pair, 1, wq_sb)
            for mm in thunks:
                mm()
            conv_qk_copy(qcT, ps, bq_sb, 0, 1)
            e_prev = e_t0

            # deferred b1 conv work, drained 2 units/tile across t1..t3
            def ck(jp):
                xt = load_xqk_pair(xk, 1, jp)
                for joff in range(2):
                    ps, thunks = conv_qk_mms(xt, joff, wk_sb)
                    for mm in thunks:
                        mm()
                    conv_qk_copy(kcT, ps, bk_sb, 1, jp * 2 + joff)
            b1_work = [lambda: ck(0), lambda: conv_v_unit(1, 0),
                       lambda: ck(1), lambda: conv_v_unit(1, 1),
                       lambda: conv_v_unit(1, 2), lambda: conv_v_unit(1, 3)]
            b1_per_tile = 2

            # ---------- steady tiles t = 1..7 + epilogue ----------
            for t in range(1, 9):
                # staggered-reset stage boundaries: stages = {pro+t1},
                # {t2,t3}, {t4,t5}, {t6,t7,epi}.  b0 stages touch only
                # b0 slices of qcT/kcT/V/y (and vice versa), so adjacent-
                # stage overlap across the back edge is data-disjoint.
                if staggered and t in (2, 4, 6):
                    tc.stage_boundary()
                b, jq = divmod(t, 4) if t < 8 else (None, None)
                e_cur = []
                bstream = attn_b_stream((t - 1) // 4, (t - 1) % 4, e_prev)

                if t < 8:
                    # conv-q(t) ran during tile t-1; here interleave
                    # conv-q(t+1) into the scores/exp loop as PE gap-filler.
                    nb, njq = divmod(t + 1, 4) if t + 1 < 8 else (None, None)
                    cq_thunks = []
                    cq_ps = None
                    if nb is not None:
                        if njq % 2 == 0:
                            xq_pair = load_xqk_pair(xq, nb, njq // 2)
                        cq_ps, cq_thunks = conv_qk_mms(xq_pair, njq % 2,
                                                       wq_sb)

                    # interleave: 16 chunk-steps
                    cqi = 0
                    for c in range(16):
                        scores_chunk(b, jq, c, e_cur)
                        # pace conv-q: 24 mms over 16 steps
                        target = (c + 1) * len(cq_thunks) // 16
                        while cqi < target:
                            cq_thunks[cqi]()
                            cqi += 1
                        # pace attn-B of t-1: 64 mm-units over 16 steps
                        units = 0
                        for kind, fn in bstream:
                            fn()
                            if kind == "mm":
                                units += 1
                                if units >= 4:
                                    break
                            # finish units don't count against the pace
                    # drain remaining B-stream units
                    for kind, fn in bstream:
                        fn()
                    if cq_ps is not None:
                        conv_qk_copy(qcT, cq_ps, bq_sb, nb, njq)
                    # b1 conv work during t1..t3
                    if t <= 3:
                        for _ in range(b1_per_tile):
                            if b1_work:
                                b1_work.pop(0)()
                    e_prev = e_cur
                else:
                    # epilogue: drain B of t7
                    for kind, fn in bstream:
                        fn()

        import os
        STAGGER = os.environ.get("KERNEL_STAGGER", "1") == "1"
        if repeat == 1:
            staggered = False
            body()
        else:
            staggered = STAGGER
            with tc.For_i(0, repeat, 1, staggered_reset=STAGGER,
                          hint_engines=(mybir.EngineType.PE,
                                        mybir.EngineType.Activation,
                                        mybir.EngineType.DVE,
                                        mybir.EngineType.SP)):
                body()

    nc.compile()
    nc.m = get_hw_module(nc.m)
    return nc


def host_prep(inputs):
    """Returns (in_maps, bias_y) — per-core input dicts + host-side bias."""
    import ml_dtypes
    bf16 = ml_dtypes.bfloat16
    q = np.asarray(inputs["q"], np.float32)
    k = np.asarray(inputs["k"], np.float32)
    v = np.asarray(inputs["v"], np.float32)
    wq_w = np.asarray(inputs["wq_w"], np.float32)
    wk_w = np.asarray(inputs["wk_w"], np.float32)
    wv_w = np.asarray(inputs["wv_w"], np.float32)
    wc_w = np.asarray(inputs["wc_w"], np.float32)

    def pad_T(x):  # [B,S,D] -> [D, B*(S+2)] zero-padded at batch edges
        out = np.zeros((D, B * SP), np.float32)
        xT = np.swapaxes(x, 1, 2)  # [B, D, S]
        for b in range(B):
            out[:, b * SP + 1: b * SP + 1 + S] = xT[b]
        return np.ascontiguousarray(out)

    xq = pad_T(q)
    xk = pad_T(k)
    xv = np.ascontiguousarray(
        np.swapaxes(v, 1, 2).transpose(1, 0, 2).reshape(D, NPOS))

    def pack_w3(w_dev):  # [128co, 1024ci, 3t] -> [p, (t c), m] = [128,24,128]
        a = w_dev.transpose(1, 2, 0)          # [ci, t, co]
        a = a.reshape(8, 128, 3, 128)         # [c, p, t, co]
        return np.ascontiguousarray(
            a.transpose(1, 2, 0, 3).reshape(128, 24, 128))

    def pack_w1(w_dev):  # [128co, 1024ci] -> [p, c, m] = [128, 8, 128]
        a = w_dev.T.reshape(8, 128, 128)      # [c, p, co]
        return np.ascontiguousarray(a.transpose(1, 0, 2))

    cast = lambda a: a.astype(bf16)
    in_maps = []
    bias_y = np.zeros((D,), np.float32)
    for dev in range(NCORES):
        heads = [2 * dev, 2 * dev + 1]
        rows = np.array([di * H + h for h in heads for di in range(HD)])
        feat = slice(2 * dev * HD, 2 * dev * HD + 128)
        wc_slice = np.ascontiguousarray(wc_w[:, feat].T)   # [128, 1024]
        bv_dev = np.asarray(inputs["wv_b"], np.float32)[rows]
        bias_y += bv_dev @ wc_slice
        in_maps.append({
            "xq": cast(xq), "xk": cast(xk), "xv": cast(xv),
            "ident": np.eye(128, dtype=bf16),
            "wq": cast(pack_w3(wq_w[rows])),
            "wk": cast(pack_w3(wk_w[rows])),
            "wv": cast(pack_w1(wv_w[rows, :, 0])),
            "wc": cast(wc_slice),
            "bq": np.ascontiguousarray(
                np.asarray(inputs["wq_b"], np.float32)[rows][:, None]),
            "bk": np.ascontiguousarray(
                np.asarray(inputs["wk_b"], np.float32)[rows][:, None]),
        })
    bias_y += np.asarray(inputs["wc_b"], np.float32)
    return in_maps, bias_y


class Runner:
    """Caches the compiled module + jitted SPMD callable (mirrors
    bass2jax.run_bass_via_pjrt, but reusable across calls)."""

    def __init__(self, repeat: int = 1):
        import jax
        from jax.sharding import Mesh, PartitionSpec
        from jax.experimental.shard_map import shard_map
        from concourse.bass2jax import (
            _bass_exec_p, install_neuronx_cc_hook, partition_id_tensor)

        self.jax = jax
        nc = build_module(repeat)
        self.nc = nc
        install_neuronx_cc_hook()
        assert nc.dbg_addr is None

        in_names, out_names, out_avals, zero_outs = [], [], [], []
        pname = nc.partition_id_tensor.name if nc.partition_id_tensor else None
        for alloc in nc.m.functions[0].allocations:
            if not isinstance(alloc, mybir.MemoryLocationSet):
                continue
            name = alloc.memorylocations[0].name
            if alloc.kind == "ExternalInput":
                if name != pname:
                    in_names.append(name)
            elif alloc.kind == "ExternalOutput":
                out_names.append(name)
                shape = tuple(alloc.tensor_shape)
                dt = mybir.dt.np(alloc.dtype)
                out_avals.append(jax.core.ShapedArray(shape, dt))
                zero_outs.append(np.zeros(shape, dt))
        self.in_names, self.out_names = in_names, out_names
        self.out_avals, self.zero_outs = out_avals, zero_outs
        n_params, n_outs = len(in_names), len(out_avals)
        all_names = in_names + out_names + ([pname] if pname else [])

        def _body(*args):
            operands = list(args)
            if pname:
                operands.append(partition_id_tensor())
            return tuple(_bass_exec_p.bind(
                *operands,
                out_avals=tuple(out_avals),
                in_names=tuple(all_names),
                out_names=tuple(out_names),
                lowering_input_output_aliases=(),
                sim_require_finite=True,
                sim_require_nnan=True,
                nc=nc))

        devices = jax.devices()[:NCORES]
        self.mesh = Mesh(np.asarray(devices), ("core",))
        self.sharded = jax.jit(
            shard_map(_body, mesh=self.mesh,
                      in_specs=(PartitionSpec("core"),) * (n_params + n_outs),
                      out_specs=(PartitionSpec("core"),) * n_outs,
                      check_rep=False),
            donate_argnums=tuple(range(n_params, n_params + n_outs)),
            keep_unused=True)

    def concat_inputs(self, in_maps):
        return [np.concatenate([np.asarray(m[n]) for m in in_maps], axis=0)
                for n in self.in_names]

    def concat_zeros(self):
        return [np.zeros((NCORES * z.shape[0], *z.shape[1:]), z.dtype)
                for z in self.zero_outs]

    def call(self, concat_in, concat_zero):
        """Returns device output arrays (not fetched)."""
        out = self.sharded(*concat_in, *concat_zero)
        self.jax.block_until_ready(out)
        return out

    def run(self, in_maps):
        out = self.call(self.concat_inputs(in_maps), self.concat_zeros())
        return [
            {n: np.asarray(out[i]).reshape(NCORES, *self.out_avals[i].shape)[c]
             for i, n in enumerate(self.out_names)}
            for c in range(NCORES)]


_CACHED = {}


def get_runner(repeat: int = 1) -> Runner:
    if repeat not in _CACHED:
        _CACHED[repeat] = Runner(repeat)
    return _CACHED[repeat]


def run(in_maps, repeat: int = 1):
    return get_runner(repeat).run(in_maps)


def kernel(**inputs) -> np.ndarray:
    in_maps, bias_y = host_prep(inputs)
    results = run(in_maps)
    y = np.zeros((NPOS, D), np.float64)
    for r in results:
        y += r["y"].astype(np.float64)
    y = y.astype(np.float32) + bias_y[None, :]
    return y.reshape(B, S, D)


if __name__ == "__main__":
    rng = np.random.default_rng(0)
    fake = {
        "q": rng.standard_normal((B, S, D)).astype(np.float32),
        "k": rng.standard_normal((B, S, D)).astype(np.float32),
        "v": rng.standard_normal((B, S, D)).astype(np.float32),
        "wq_w": (rng.standard_normal((D, D, 3)) / 32).astype(np.float32),
        "wq_b": np.zeros(D, np.float32),
        "wk_w": (rng.standard_normal((D, D, 3)) / 32).astype(np.float32),
        "wk_b": np.zeros(D, np.float32),
        "wv_w": (rng.standard_normal((D, D, 1)) / 32).astype(np.float32),
        "wv_b": np.zeros(D, np.float32),
        "wc_w": (rng.standard_normal((D, D)) / 32).astype(np.float32),
        "wc_b": np.zeros(D, np.float32),
    }
    out = kernel(**fake)
    print("kernel output", out.shape, out.dtype)


# revision 28
# speedup vs baseline: 81.9195x; 1.0239x over previous
"""ConvMultiHeadAttention Trainium2 kernel (8-core SPMD, batch+head sharded).

Module: conv1d(k=3,pad=1) Q/K proj, conv1d(k=1) V proj, 16-head attention
(head = channel%16), concat, linear out-proj.  B=2, S=2048, D=1024, d=64.

Sharding: each of the 8 cores owns 2 heads x both batches.  Conv weights are
row-sliced per core (128 output channels each, ordered [head0 d0..63,
head1 d0..63]); q/k/v inputs are replicated (conv contracts all 1024 input
channels).  Each core produces a y-partial [4096, 1024] = (its heads' attn
output) @ wc_slice^T; the host sums the 8 partials and adds the biases that
commute out (wc_b, and bv @ wc_slice^T since softmax weights sum to 1).

v2 design (all matmuls bf16 = 1 cycle/row on PE; v1 was fp32 = 4):
  conv q/k  -> qcT/kcT [128ch, 4096pos] bf16 in SBUF; bias added by DVE
              tensor_scalar during the psum->sbuf copy.
  conv v    -> V0/V1 [128key, 32chunk, 65] bf16, col 64 memset to 1.0: the
              ones column makes attn@V also accumulate the softmax
              denominator r = sum_k E[k,q] for free.
  scores^T  sp[128key, 2head, 512q] psum (2 banks, one accumulation group
              per bank); exp -> E bf16 [128,2,512] on ACT (scale=1/8).
  attn@V    out[q,d] orientation: lhsT = E chunk [128k,128q], rhs =
              V[128k, 65] -> psum [128q, 65]; accumulate 16 chunks; full
              128x128 PE utilization (v1 orientation wasted half).
  softmax   rinv = 1/psum[:,64] (DVE); normalize fused into the psum->sbuf
              copy via DVE tensor_scalar_mul with per-partition scalar.
  transpose [q,feat] -> [feat,q] on PE (identity matmul through the proj
              psum slot).  DMA-XBAR transposes measured ~2.6us/op through
              this fabric (per-op DMA overhead) — PE is ~30x cheaper here.
  proj      yp[128q, 512od] = outT.T @ wc (bf16); DVE copy -> y_sb bf16;
              ONE batched [512,1024] y store per 512-position tile.

DMA op count is deliberately tiny (~24/iter): x loads are 1026-wide pairs
serving two conv tiles, xv tiles are 512-wide, y stores batch 4 qgroups.
(A 96-op/iter variant measured +2.6us of per-op overhead per DMA on HW.)

Emission is software-pipelined: tile t's scores/exp loop interleaves
conv-q of tile t+1 (fine-grained, PE gap-filler while ACT drains exp) and
the full attention-B of tile t-1 (attn@V + normalize + transpose + proj),
so PE and ACT both stay ~busy.  PSUM budget (8 banks): scores 2x[128,2,512]
(4) + conv [128,512] (1) + at0/at1 [128,65] (2) + transpose/proj [128,512]
(1).  Plain For_i for the repeat loop: staggered_reset measured 18us/iter
SLOWER (its no-sync stage barriers cost more than the back-edge they save).
"""

import sys
import numpy as np
from contextlib import ExitStack

sys.path.insert(0, "/opt/trn_rl_repo")

import concourse.bass as bass
import concourse.tile as tile
from concourse import bacc, mybir
from concourse.bass_interp import get_hw_module
from concourse import bass2jax

F32 = mybir.dt.float32
BF16 = mybir.dt.bfloat16

NCORES = 8
B, S, D = 2, 2048, 1024
H, HD = 16, 64          # heads, head dim
CO = 128                # conv output channels per core (2 heads x 64)
SP = S + 2              # padded positions per batch for k=3 conv
NPOS = B * S            # 4096
NCHUNK = NPOS // 128    # 32 key chunks


def build_module(repeat: int = 1):
    nc = bacc.Bacc("TRN2", target_bir_lowering=False, debug=False,
                   num_devices=NCORES)

    xq = nc.dram_tensor("xq", [D, B * SP], BF16, kind="ExternalInput").ap()
    xk = nc.dram_tensor("xk", [D, B * SP], BF16, kind="ExternalInput").ap()
    xv = nc.dram_tensor("xv", [D, NPOS], BF16, kind="ExternalInput").ap()
    wq = nc.dram_tensor("wq", [128, 24, 128], BF16, kind="ExternalInput").ap()
    wk = nc.dram_tensor("wk", [128, 24, 128], BF16, kind="ExternalInput").ap()
    wv = nc.dram_tensor("wv", [128, 8, 128], BF16, kind="ExternalInput").ap()
    wc = nc.dram_tensor("wc", [128, 1024], BF16, kind="ExternalInput").ap()
    bq = nc.dram_tensor("bq", [128, 1], F32, kind="ExternalInput").ap()
    bk = nc.dram_tensor("bk", [128, 1], F32, kind="ExternalInput").ap()
    ident = nc.dram_tensor("ident", [128, 128], BF16,
                           kind="ExternalInput").ap()
    y = nc.dram_tensor("y", [NPOS, D], BF16, kind="ExternalOutput").ap()

    with tile.TileContext(nc) as tc, ExitStack() as ctx:
        wpool = ctx.enter_context(tc.tile_pool(name="wpool", bufs=1))
        cpool = ctx.enter_context(tc.tile_pool(name="cpool", bufs=1))
        xpool = ctx.enter_context(tc.tile_pool(name="xpool", bufs=3))
        epool = ctx.enter_context(tc.tile_pool(name="epool", bufs=34))
        spool = ctx.enter_context(tc.tile_pool(name="spool", bufs=2))
        ypool = ctx.enter_context(tc.tile_pool(name="ypool", bufs=3))

        # ---- persistent weights (ACT hwdge queue, so the first x-tile
        # loads on the SP queue aren't stuck behind them) ----
        wq_sb = wpool.tile([128, 24, 128], BF16)
        nc.scalar.dma_start(wq_sb[:], wq[:])
        wk_sb = wpool.tile([128, 24, 128], BF16)
        wv_sb = wpool.tile([128, 8, 128], BF16)
        wc_sb = wpool.tile([128, 1024], BF16)
        bq_sb = wpool.tile([128, 1], F32)
        bk_sb = wpool.tile([128, 1], F32)
        nc.scalar.dma_start(wk_sb[:], wk[:])
        nc.scalar.dma_start(wv_sb[:], wv[:])
        nc.scalar.dma_start(wc_sb[:], wc[:])
        nc.scalar.dma_start(bq_sb[:], bq[:])
        nc.scalar.dma_start(bk_sb[:], bk[:])
        ident_sb = wpool.tile([128, 128], BF16)
        nc.scalar.dma_start(ident_sb[:], ident[:])

        # ---- persistent activations ----
        qcT = cpool.tile([128, NPOS], BF16)
        kcT = cpool.tile([128, NPOS], BF16)
        V0 = cpool.tile([128, NCHUNK, 65], BF16)   # head0 V + ones col
        V1 = cpool.tile([128, NCHUNK, 65], BF16)   # head1 V + ones col
        nc.vector.memset(V0[:, :, 64:65], 1.0)
        nc.vector.memset(V1[:, :, 64:65], 1.0)

        # PSUM pools: sp 2x[128,2,512] (4 banks) + cq [128,512] (1) +
        # at0/at1 [128,65] (2) + yp [128,512] (1) = 8 banks exactly.
        psp = ctx.enter_context(tc.tile_pool(name="psp", bufs=2, space="PSUM"))
        pcq = ctx.enter_context(tc.tile_pool(name="pcq", bufs=1, space="PSUM"))
        pat = ctx.enter_context(tc.tile_pool(name="pat", bufs=1, space="PSUM"))
        pyp = ctx.enter_context(tc.tile_pool(name="pyp", bufs=1, space="PSUM"))

        def load_xqk_pair(src, b, jp):
            """One 1026-wide load serving two adjacent 512-col conv tiles."""
            xt = xpool.tile([128, 8, 1026], BF16, tag="xqk", bufs=2)
            col0 = b * SP + jp * 1024
            nc.sync.dma_start(
                xt[:],
                src[:, col0:col0 + 1026].rearrange("(c p) i -> p c i", p=128))
            return xt

        def conv_qk_mms(xt, joff, w_sb):
            """Yields the 24 accumulating matmul thunks for one 512-col tile
            (slice joff in {0,1} of a pair-load); the caller paces them."""
            ps = pcq.tile([128, 512], F32, tag="cq")
            thunks = []
            for t in range(3):
                for c in range(8):
                    n = t * 8 + c
                    o = joff * 512
                    def mm(n=n, t=t, c=c, o=o, ps=ps, xt=xt, w_sb=w_sb):
                        nc.tensor.matmul(ps[:], w_sb[:, n, :],
                                         xt[:, c, o + t:o + t + 512],
                                         start=(n == 0), stop=(n == 23))
                    thunks.append(mm)
            return ps, thunks

        def conv_qk_copy(outT, ps, b_sb, b, j):
            cols = slice(b * S + j * 512, b * S + (j + 1) * 512)
            nc.vector.tensor_scalar_add(outT[:, cols], ps[:], b_sb[:, 0:1])

        def conv_v_unit(b, j):
            """One 512-position V tile: load, 32 matmuls, 2 copies."""
            xt = xpool.tile([128, 8, 512], BF16, tag="xv", bufs=2)
            col0 = b * S + j * 512
            nc.sync.dma_start(
                xt[:],
                xv[:, col0:col0 + 512].rearrange("(c p) i -> p c i", p=128))
            vp = pcq.tile([128, 4, 128], F32, tag="cq")
            for g in range(4):
                for c in range(8):
                    nc.tensor.matmul(vp[:, g, :], xt[:, c, g * 128:(g + 1) * 128],
                                     wv_sb[:, c, :],
                                     start=(c == 0), stop=(c == 7))
            c0 = b * 16 + j * 4
            nc.vector.tensor_copy(V0[:, c0:c0 + 4, 0:64], vp[:, :, 0:64])
            nc.vector.tensor_copy(V1[:, c0:c0 + 4, 0:64], vp[:, :, 64:128])

        def scores_chunk(b, jq, c, e_tiles):
            """Scores + exp for key-chunk c of tile (b, jq)."""
            q0 = b * S + jq * 512
            k0 = b * S + c * 128
            sp = psp.tile([128, 2, 512], F32, tag="sp")
            nc.tensor.matmul(sp[:, 0, :], kcT[0:64, k0:k0 + 128],
                             qcT[0:64, q0:q0 + 512], start=True, stop=True)
            nc.tensor.matmul(sp[:, 1, :], kcT[64:128, k0:k0 + 128],
                             qcT[64:128, q0:q0 + 512], start=True, stop=True)
            e = epool.tile([128, 2, 512], BF16, tag="e")
            nc.scalar.activation(e[:], sp[:],
                                 mybir.ActivationFunctionType.Exp, scale=0.125)
            e_tiles.append(e)

        def attn_b_stream(b, jq, e_tiles):
            """Yields work units for the B-phase of tile (b, jq): per qgroup
            32 attn@V matmul thunks, a normalize+transpose thunk, and —
            lagged one qgroup so attn@V matmuls hide the single-bank proj
            ping-pong — a proj+store thunk."""
            outT_by_qg = {}
            y_tile_box = [None]
            pending_proj = None
            for qg in range(4):
                at0 = pat.tile([128, 65], F32, tag="at0")
                at1 = pat.tile([128, 65], F32, tag="at1")
                qs = slice(qg * 128, (qg + 1) * 128)
                for c in range(16):
                    cb = b * 16 + c
                    def mm(c=c, cb=cb, at0=at0, at1=at1, qs=qs):
                        e = e_tiles[c]
                        nc.tensor.matmul(at0[:], e[:, 0, qs], V0[:, cb, :],
                                         start=(c == 0), stop=(c == 15))
                        nc.tensor.matmul(at1[:], e[:, 1, qs], V1[:, cb, :],
                                         start=(c == 0), stop=(c == 15))
                    yield ("mm", mm)
                    if c == 7 and pending_proj is not None:
                        yield ("finish", pending_proj)
                        pending_proj = None

                def norm_tr(qg=qg, at0=at0, at1=at1):
                    norm = spool.tile([128, 128], BF16, tag="norm", bufs=2)
                    outT = spool.tile([128, 128], BF16, tag="outT", bufs=3)
                    for h, at in ((0, at0), (1, at1)):
                        rinv = spool.tile([128, 1], F32, tag="rinv", bufs=4)
                        nc.vector.reciprocal(rinv[:], at[:, 64:65])
                        nc.vector.tensor_scalar_mul(
                            norm[:, h * 64:(h + 1) * 64],
                            at[:, 0:64], rinv[:, 0:1])
                    # [q, feat] -> [feat, q]: PE transpose through the proj
                    # psum slot (DMA-XBAR transposes cost ~2.6us each in
                    # per-op DMA overhead on this fabric)
                    trp = pyp.tile([128, 128], BF16, tag="yp")
                    nc.tensor.transpose(trp[:], norm[:], ident_sb[:])
                    nc.vector.tensor_copy(outT[:], trp[:])
                    outT_by_qg[qg] = outT
                yield ("finish", norm_tr)

                def proj(qg=qg, b=b, jq=jq):
                    outT = outT_by_qg.pop(qg)
                    if qg == 0:
                        y_new = ypool.tile([128, 4, 1024], BF16,
                                           tag="ysb", bufs=2)
                        y_tile_box[0] = y_new
                    y_sb = y_tile_box[0]
                    for half in range(2):
                        yp = pyp.tile([128, 512], F32, tag="yp")
                        nc.tensor.matmul(
                            yp[:], outT[:],
                            wc_sb[:, half * 512:(half + 1) * 512],
                            start=True, stop=True)
                        nc.vector.tensor_copy(
                            y_sb[:, qg, half * 512:(half + 1) * 512], yp[:])
                    if qg == 3:
                        row0 = b * S + jq * 512
                        nc.sync.dma_start(
                            y[row0:row0 + 512, :].rearrange(
                                "(j p) d -> p j d", p=128), y_sb[:])
                pending_proj = proj
            yield ("finish", pending_proj)

        def body():
            # ---------- prologue: conv q(0,0)+(0,1), conv k b0 + scores t0 --
            xq_pair = load_xqk_pair(xq, 0, 0)   # serves conv-q of tiles 0,1
            ps, thunks = conv_qk_mms(xq_pair, 0, wq_sb)
            for mm in thunks:
                mm()
            conv_qk_copy(qcT, ps, bq_sb, 0, 0)
            e_t0 = []
            for jp in range(2):
                xt = load_xqk_pair(xk, 0, jp)
                for joff in range(2):
                    j = jp * 2 + joff
                    ps, thunks = conv_qk_mms(xt, joff, wk_sb)
                    for mm in thunks:
                        mm()
                    conv_qk_copy(kcT, ps, bk_sb, 0, j)
                    for c in range(4 * j, 4 * j + 4):
                        scores_chunk(0, 0, c, e_t0)
            for j in range(4):
                conv_v_unit(0, j)
            # conv q for tile 1 (b0, jq1) — tile t's conv-q runs in tile t-1
            ps, thunks = conv_qk_mms(xq_pair, 1, wq_sb)
            for mm in thunks:
                mm()
            conv_qk_copy(qcT, ps, bq_sb, 0, 1)
            e_prev = e_t0

            # deferred b1 conv work, drained 2 units/tile across t1..t3
            def ck(jp):
                xt = load_xqk_pair(xk, 1, jp)
                for joff in range(2):
                    ps, thunks = conv_qk_mms(xt, joff, wk_sb)
                    for mm in thunks:
                        mm()
                    conv_qk_copy(kcT, ps, bk_sb, 1, jp * 2 + joff)
            b1_work = [lambda: ck(0), lambda: conv_v_unit(1, 0),
                       lambda: ck(1), lambda: conv_v_unit(1, 1),
                       lambda: conv_v_unit(1, 2), lambda: conv_v_unit(1, 3)]
            b1_per_tile = 2

            # ---------- steady tiles t = 1..7 + epilogue ----------
            for t in range(1, 9):
                # staggered-reset stage boundaries: stages = {pro+t1},
                # {t2,t3}, {t4,t5}, {t6,t7,epi}.  b0 stages touch only
                # b0 slices of qcT/kcT/V/y (and vice versa), so adjacent-
                # stage overlap across the back edge is data-disjoint.
                if staggered and t in (2, 4, 6):
                    tc.stage_boundary()
                b, jq = divmod(t, 4) if t < 8 else (None, None)
                e_cur = []
                bstream = attn_b_stream((t - 1) // 4, (t - 1) % 4, e_prev)

                if t < 8:
                    # conv-q(t) ran during tile t-1; here interleave
                    # conv-q(t+1) into the scores/exp loop as PE gap-filler.
                    nb, njq = divmod(t + 1, 4) if t + 1 < 8 else (None, None)
                    cq_thunks = []
                    cq_ps = None
                    if nb is not None:
                        if njq % 2 == 0:
                            xq_pair = load_xqk_pair(xq, nb, njq // 2)
                        cq_ps, cq_thunks = conv_qk_mms(xq_pair, njq % 2,
                                                       wq_sb)

                    # interleave: 16 chunk-steps
                    cqi = 0
                    for c in range(16):
                        scores_chunk(b, jq, c, e_cur)
                        # pace conv-q: 24 mms over 16 steps
                        target = (c + 1) * len(cq_thunks) // 16
                        while cqi < target:
                            cq_thunks[cqi]()
                            cqi += 1
                        # pace attn-B of t-1: 64 mm-units over 16 steps
                        units = 0
                        for kind, fn in bstream:
                            fn()
                            if kind == "mm":
                                units += 1
                                if units >= 4:
                                    break
                            # finish units don't count against the pace
                    # drain remaining B-stream units
                    for kind, fn in bstream:
                        fn()
                    if cq_ps is not None:
                        conv_qk_copy(qcT, cq_ps, bq_sb, nb, njq)
                    # b1 conv work during t1..t3
                    if t <= 3:
                        for _ in range(b1_per_tile):
                            if b1_work:
                                b1_work.pop(0)()
                    e_prev = e_cur
                else:
                    # epilogue: drain B of t7
                    for kind, fn in bstream:
                        fn()

        import os
        STAGGER = os.environ.get("KERNEL_STAGGER", "0") == "1"
        if repeat == 1:
            staggered = False
            body()
        else:
            staggered = STAGGER
            with tc.For_i(0, repeat, 1, staggered_reset=STAGGER,
                          hint_engines=(mybir.EngineType.PE,
                                        mybir.EngineType.Activation,
                                        mybir.EngineType.DVE,
                                        mybir.EngineType.SP)):
                body()

    nc.compile()
    nc.m = get_hw_module(nc.m)
    return nc


def host_prep(inputs):
    """Returns (in_maps, bias_y) — per-core input dicts + host-side bias."""
    import ml_dtypes
    bf16 = ml_dtypes.bfloat16
    q = np.asarray(inputs["q"], np.float32)
    k = np.asarray(inputs["k"], np.float32)
    v = np.asarray(inputs["v"], np.float32)
    wq_w = np.asarray(inputs["wq_w"], np.float32)
    wk_w = np.asarray(inputs["wk_w"], np.float32)
    wv_w = np.asarray(inputs["wv_w"], np.float32)
    wc_w = np.asarray(inputs["wc_w"], np.float32)

    def pad_T(x):  # [B,S,D] -> [D, B*(S+2)] zero-padded at batch edges
        out = np.zeros((D, B * SP), np.float32)
        xT = np.swapaxes(x, 1, 2)  # [B, D, S]
        for b in range(B):
            out[:, b * SP + 1: b * SP + 1 + S] = xT[b]
        return np.ascontiguousarray(out)

    xq = pad_T(q)
    xk = pad_T(k)
    xv = np.ascontiguousarray(
        np.swapaxes(v, 1, 2).transpose(1, 0, 2).reshape(D, NPOS))

    def pack_w3(w_dev):  # [128co, 1024ci, 3t] -> [p, (t c), m] = [128,24,128]
        a = w_dev.transpose(1, 2, 0)          # [ci, t, co]
        a = a.reshape(8, 128, 3, 128)         # [c, p, t, co]
        return np.ascontiguousarray(
            a.transpose(1, 2, 0, 3).reshape(128, 24, 128))

    def pack_w1(w_dev):  # [128co, 1024ci] -> [p, c, m] = [128, 8, 128]
        a = w_dev.T.reshape(8, 128, 128)      # [c, p, co]
        return np.ascontiguousarray(a.transpose(1, 0, 2))

    cast = lambda a: a.astype(bf16)
    in_maps = []
    bias_y = np.zeros((D,), np.float32)
    for dev in range(NCORES):
        heads = [2 * dev, 2 * dev + 1]
        rows = np.array([di * H + h for h in heads for di in range(HD)])
        feat = slice(2 * dev * HD, 2 * dev * HD + 128)
        wc_slice = np.ascontiguousarray(wc_w[:, feat].T)   # [128, 1024]
        bv_dev = np.asarray(inputs["wv_b"], np.float32)[rows]
        bias_y += bv_dev @ wc_slice
        in_maps.append({
            "xq": cast(xq), "xk": cast(xk), "xv": cast(xv),
            "ident": np.eye(128, dtype=bf16),
            "wq": cast(pack_w3(wq_w[rows])),
            "wk": cast(pack_w3(wk_w[rows])),
            "wv": cast(pack_w1(wv_w[rows, :, 0])),
            "wc": cast(wc_slice),
            "bq": np.ascontiguousarray(
                np.asarray(inputs["wq_b"], np.float32)[rows][:, None]),
            "bk": np.ascontiguousarray(
                np.asarray(inputs["wk_b"], np.float32)[rows][:, None]),
        })
    bias_y += np.asarray(inputs["wc_b"], np.float32)
    return in_maps, bias_y


class Runner:
    """Caches the compiled module + jitted SPMD callable (mirrors
    bass2jax.run_bass_via_pjrt, but reusable across calls)."""

    def __init__(self, repeat: int = 1):
        import jax
        from jax.sharding import Mesh, PartitionSpec
        from jax.experimental.shard_map import shard_map
        from concourse.bass2jax import (
            _bass_exec_p, install_neuronx_cc_hook, partition_id_tensor)

        self.jax = jax
        nc = build_module(repeat)
        self.nc = nc
        install_neuronx_cc_hook()
        assert nc.dbg_addr is None

        in_names, out_names, out_avals, zero_outs = [], [], [], []
        pname = nc.partition_id_tensor.name if nc.partition_id_tensor else None
        for alloc in nc.m.functions[0].allocations:
            if not isinstance(alloc, mybir.MemoryLocationSet):
                continue
            name = alloc.memorylocations[0].name
            if alloc.kind == "ExternalInput":
                if name != pname:
                    in_names.append(name)
            elif alloc.kind == "ExternalOutput":
                out_names.append(name)
                shape = tuple(alloc.tensor_shape)
                dt = mybir.dt.np(alloc.dtype)
                out_avals.append(jax.core.ShapedArray(shape, dt))
                zero_outs.append(np.zeros(shape, dt))
        self.in_names, self.out_names = in_names, out_names
        self.out_avals, self.zero_outs = out_avals, zero_outs
        n_params, n_outs = len(in_names), len(out_avals)
        all_names = in_names + out_names + ([pname] if pname else [])

        def _body(*args):
            operands = list(args)
            if pname:
                operands.append(partition_id_tensor())
            return tuple(_bass_exec_p.bind(
                *operands,
                out_avals=tuple(out_avals),
                in_names=tuple(all_names),
                out_names=tuple(out_names),
                lowering_input_output_aliases=(),
                sim_require_finite=True,
                sim_require_nnan=True,
                nc=nc))

        devices = jax.devices()[:NCORES]
        self.mesh = Mesh(np.asarray(devices), ("core",))
        self.sharded = jax.jit(
            shard_map(_body, mesh=self.mesh,
                      in_specs=(PartitionSpec("core"),) * (n_params + n_outs),
                      out_specs=(PartitionSpec("core"),) * n_outs,
                      check_rep=False),
            donate_argnums=tuple(range(n_params, n_params + n_outs)),
            keep_unused=True)

    def concat_inputs(self, in_maps):
        return [np.concatenate([np.asarray(m[n]) for m in in_maps], axis=0)
                for n in self.in_names]

    def concat_zeros(self):
        return [np.zeros((NCORES * z.shape[0], *z.shape[1:]), z.dtype)
                for z in self.zero_outs]

    def call(self, concat_in, concat_zero):
        """Returns device output arrays (not fetched)."""
        out = self.sharded(*concat_in, *concat_zero)
        self.jax.block_until_ready(out)
        return out

    def run(self, in_maps):
        out = self.call(self.concat_inputs(in_maps), self.concat_zeros())
        return [
            {n: np.asarray(out[i]).reshape(NCORES, *self.out_avals[i].shape)[c]
             for i, n in enumerate(self.out_names)}
            for c in range(NCORES)]


_CACHED = {}


def get_runner(repeat: int = 1) -> Runner:
    if repeat not in _CACHED:
        _CACHED[repeat] = Runner(repeat)
    return _CACHED[repeat]


def run(in_maps, repeat: int = 1):
    return get_runner(repeat).run(in_maps)


def kernel(**inputs) -> np.ndarray:
    in_maps, bias_y = host_prep(inputs)
    results = run(in_maps)
    y = np.zeros((NPOS, D), np.float64)
    for r in results:
        y += r["y"].astype(np.float64)
    y = y.astype(np.float32) + bias_y[None, :]
    return y.reshape(B, S, D)


if __name__ == "__main__":
    rng = np.random.default_rng(0)
    fake = {
        "q": rng.standard_normal((B, S, D)).astype(np.float32),
        "k": rng.standard_normal((B, S, D)).astype(np.float32),
        "v": rng.standard_normal((B, S, D)).astype(np.float32),
        "wq_w": (rng.standard_normal((D, D, 3)) / 32).astype(np.float32),
        "wq_b": np.zeros(D, np.float32),
        "wk_w": (rng.standard_normal((D, D, 3)) / 32).astype(np.float32),
        "wk_b": np.zeros(D, np.float32),
        "wv_w": (rng.standard_normal((D, D, 1)) / 32).astype(np.float32),
        "wv_b": np.zeros(D, np.float32),
        "wc_w": (rng.standard_normal((D, D)) / 32).astype(np.float32),
        "wc_b": np.zeros(D, np.float32),
    }
    out = kernel(**fake)
    print("kernel output", out.shape, out.dtype)
